# revision 1
# baseline (speedup 1.0000x reference)
"""Distributed brute-force kNN (top-50 inverse-distance-weighted regression), v2.

Strategy (8 NeuronCores):
  - Table (500k x 64) norm-sorted and striped across 8 cores (62500 rows each,
    padded to 63488 = 62 units x 1024 cols). fp8e4m3 everywhere on device;
    dots via DoubleRow matmuls (K packed as [32 partitions x 2 k-tiles],
    0.5 cycles/row => 2x bf16 PE throughput).
  - Per 128-query group, each unit's 1024 dots reduce to 128 octet maxima
    (octet = 8 consecutive-norm rows, member-major device layout:
    col = member*128 + octet). Three balanced paths:
      R  (25/62): DVE tensor_reduce straight from PSUM.
      A1 (12/62): ACT evicts PSUM -> bf16 SBUF, DVE 3-fold member halves.
      A2 (25/62): ACT evicts, Pool (gpsimd) 3-fold (gpsimd cannot touch PSUM).
  - gm columns are grouped by reducing engine (DVE units first, Pool units
    after) so each of the two per-qg output DMAs has a single writer engine
    (walrus accepts at most ONE semaphore wait per instruction; Tile has no
    transitive clock tracking, so multi-writer DMA sources are illegal).
  - Host: rank octets by the distance lower bound min_norm - 2*gmax, take
    top N_GROUPS=192 (fp8 dot noise measured: worst true-top-50 octet depth
    89 on the seed-0 data), re-score the 8*N_GROUPS candidates exactly in
    f32, final top-50 + inverse-distance weights (reference formula).

Sync discipline: per-engine total order via nosync deps (link()); every
cross-engine hazard is carried by exactly one instruction wait, with
absorber ops (standalone PE ldweights, tiny ACT/DVE copies) pulling engine
clocks forward so Tile never emits a second wait.
"""

import numpy as np
import ml_dtypes

import concourse.bass as bass
import concourse.tile as tile
from concourse import mybir
from concourse.bass_utils import run_bass_kernel_spmd
from contextlib import ExitStack

# Problem geometry (hardcoded per spec).
B = 1024          # queries
D = 64            # feature dim
C = 500000        # table capacity
K = 50            # neighbours
DELTA = 1e-3

CORES = 8
N_TILE = 512      # matmul free dim (one PSUM bank)
PAIR_W = 2 * N_TILE
N_UNITS = 62      # 62 * 1024 = 63488 >= 62500 real rows per core
N_COLS = N_UNITS * PAIR_W
C_LOC = 63488     # table width in tq (= N_COLS exactly)
G = 8             # octet size
OCT_W = PAIR_W // G            # 128 octets per unit
N_OCT = N_UNITS * OCT_W        # 7936 octets per core
P = 128           # partition dim == query-group size
QG = B // P       # 8 query groups

N_GROUPS = 192    # candidate octets per query taken on host

# unit kinds: R = DVE tensor_reduce (1 gm col/octet), A1 = ACT evict + 2 DVE
# folds (2 cols/octet, host maxes the pair), A2 = ACT evict + 2 Pool folds
# (2 cols/octet). Starts with R (gm col 0 must be DVE-written: the
# qg-boundary DVE copy reads prev gm[0,0] without a cross-engine wait).
def _mk_units(n_r=25, n_a1=37, n_a2=0):
    # largest-remainder interleave for smooth engine load
    counts = {k: v for k, v in
              (("R", n_r), ("A1", n_a1), ("A2", n_a2)) if v > 0}
    acc = {k: 0.0 for k in counts}
    out = []
    for i in range(N_UNITS):
        for k in counts:
            acc[k] += counts[k] / N_UNITS
        k = max(acc, key=lambda kk: acc[kk])
        acc[k] -= 1.0
        out.append(k)
    # rotate so the sequence starts with an R unit (gm col 0 DVE-written)
    i0 = out.index("R")
    out = out[i0:] + out[:i0]
    assert out[0] == "R" and all(out.count(k) == counts.get(k, 0) for k in ("R", "A1", "A2"))
    return out

UNITS = _mk_units()
N_R = UNITS.count("R")
N_A1 = UNITS.count("A1")
N_A2 = UNITS.count("A2")
DVE_COLS = N_R * OCT_W + N_A1 * 4 * OCT_W   # R: 128 cols, A1: 512 (fold1 only)
POOL_COLS = 0
GM_W = DVE_COLS + 1                         # gm row width (+1 sync pad col)

# quadrant layout: table quarter qt (16 units) lives on partitions
# [32qt, 32qt+32) with its own query replica; every unit's matmuls use
# PE row-group tiling (tile_position row base = 32qt). This packs the fp8
# k-tile layout into ~35KB/partition instead of 129KB on 32 partitions.
UQ = 21                            # units per third (PE row-group bases
NQUAD = 3                          # 0/32/64; base 96 = broken HW quadrant)
NT_Q = 2 + 2 * UQ                  # 44 512-col tiles per third (2 = queries)
# load lanes: 6 chunks = (third, half); four chunks on the two physical
# HWDGE rings (SP, ACT), two on the SWDGE queue. 4 hwdge loads +
# 4 out-DMAs = 8 = NUM_HWDGE_SEMS, so no sem reuse.
CHUNK_T = NT_Q // 4                # 11 tiles per chunk

_NC_CACHE = {}


def _build_nc():
    if "nc" in _NC_CACHE:
        return _NC_CACHE["nc"]
    nc = bass.Bass()
    # queries FIRST so the first weight loads only need the first DMA chunk
    # k-tiles interleaved per 512-col tile so every AP's k-step is 512
    # (the ISA static-pattern step field is 16-bit signed)
    tq = nc.declare_dram_parameter("tq", [96, NT_Q, 2, N_TILE],
                                   mybir.dt.float8e4, isOutput=False)
    gm = nc.declare_dram_parameter("gm", [B, GM_W], mybir.dt.bfloat16, isOutput=True)

    pad_col = DVE_COLS             # sync-pad column in gm_tile (not DMA'd)
    last_dve_col = DVE_COLS - 1    # last DVE-group gm column

    from concourse.bass import _add_dep_helper as dep
    import concourse.tile_sem_assignment as _tsa
    _tsa.NUM_SWDGE_GLOBAL_SEMS = 1

    # Split TileContext's exit drain (one wait per active proc) into
    # one-wait-per-drain instructions.
    from concourse.vector_clock import VectorClock, ScopedClock

    def _split_drain_and_barrier(self, tick_clock, wait_clock):
        gc = tick_clock.global_clock
        for proc in range(27):
            t = gc.peek_next(proc) - 1
            if t <= 0:
                continue
            d = self.nc.sync.drain()
            pc = VectorClock()
            pc.require_at_least(proc, t)
            wait_clock.add_sem_waits(d.ins, ScopedClock({None: pc}))
        self.nc.all_engine_barrier()
        assert self.sems is not None
        popped = self.nc._tile_sem_poison_stack.pop()
        assert popped is self._sem_poison
        self.nc.clear_and_free_semaphores(list(self.sems.allocated().values()))
        self.nc.all_engine_barrier()

    tile.TileContext._drain_and_barrier = _split_drain_and_barrier

    with ExitStack() as ctx:
        tc = ctx.enter_context(tile.TileContext(nc, pool_alloc_mode="queue"))
        singles = ctx.enter_context(tc.tile_pool(name="singles", bufs=1))
        # one PSUM pool per consumer engine: within a tag all releases come
        # from one engine in issue order, so slot reuse is deterministic
        ppoolA = ctx.enter_context(tc.tile_pool(name="ppoolA", bufs=2, space="PSUM"))
        ppoolR = ctx.enter_context(tc.tile_pool(name="ppoolR", bufs=2, space="PSUM"))
        spoolD = ctx.enter_context(tc.tile_pool(name="spoolD", bufs=7))
        # double-width gm tiles: one tile covers a qg PAIR, shipped by ONE
        # hwdge DMA per pair (8 hwdge DMAs total = NUM_HWDGE_SEMS, so no
        # hwdge sem is reused and no DMA needs a second queue wait)
        gmpool = ctx.enter_context(tc.tile_pool(name="gmpool", bufs=2))
        apool = ctx.enter_context(tc.tile_pool(name="apool", bufs=2))

        # chunked table load: compute starts after the first chunk instead of
        # serializing behind the whole transfer. Single SW queue keeps chunk
        # ordering for free.
        t_sb = singles.tile([96, NT_Q, 2, N_TILE], mybir.dt.float8e4)
        # chunk index = 2*third + half; all on the SWDGE queue so the 8
        # hwdge sems are reserved for the 8 per-qg output DMAs
        for ck in range(12):
            qt, h = ck // 4, ck % 4
            ps = slice(32 * qt, 32 * (qt + 1))
            ts = slice(h * CHUNK_T, (h + 1) * CHUNK_T)
            nc.gpsimd.dma_start(out=t_sb[ps, ts], in_=tq[ps, ts])

        chain = {"PE": None, "DVE": None, "ACT": None}

        def link(key, inst):
            # total order per engine queue (add_dep_helper(a, b) == a waits b)
            if chain[key] is not None:
                dep(inst.ins, chain[key].ins, sync=False, reason=f"{key} order")
            chain[key] = inst
            return inst

        a1_scr = apool.tile([1, 1], mybir.dt.bfloat16, tag="a1")
        tA1a = apool.tile([1, 1], mybir.dt.bfloat16, tag="tA1a")
        tA1b = apool.tile([1, 1], mybir.dt.bfloat16, tag="tA1b")
        tP1 = apool.tile([1, 1], mybir.dt.bfloat16, tag="tP1")
        tA2 = apool.tile([1, 1], mybir.dt.bfloat16, tag="tA2")
        act_end = apool.tile([1, 1], mybir.dt.bfloat16, tag="act_end")
        tA1s = [tA1a, tA1b]

        prev_gms = []
        a_srcs = []            # [64,1] s-tile AP per A-unit (PE absorber source)
        r_srcs = []            # [64,1] gm-col AP per R-unit
        dve_gm_cols = []       # (gm_tile, col) most recent DVE-written gm col
        pool_gm_cols = []      # (gm_tile, col) most recent Pool-written gm col
        n_a = 0                # global A-unit counter (A1+A2)
        n_r = 0                # global R-unit counter
        last_m2p = None        # most recent Pool m2 tile (boundary absorber src)

        absorbed_ck = 0
        for qg in range(QG):
            qq = (qg % 4) * P
            if qg >= 1:
                # chain-covered DVE read of the qg-1 tile: makes it the
                # latest toucher, so the recycle at qg+1 emits no DVE wait
                link("DVE", nc.vector.tensor_copy(
                    a1_scr, prev_gms[-1][0:1, last_dve_col:last_dve_col + 1]))
            gm_tile = gmpool.tile([P, DVE_COLS + 1], mybir.dt.bfloat16,
                                  tag="gmT")
            if qg >= 2:
                # first write to the recycled tile: the pad-col copy reads the
                # (alive) qg-1 tile, so its only emitted wait is the
                # out-DMA(qg-2) WAR; later writers are covered by DVE's clock
                pg = prev_gms[-1]
                link("DVE", nc.vector.tensor_copy(
                    gm_tile[0:1, pad_col:pad_col + 1], pg[0:1, 0:1]))

            dve_base = 0
            pool_base = 0
            for u, kind in enumerate(UNITS):
                qt = u // UQ            # quarter (partition base 32*qt)
                ps = slice(32 * qt, 32 * (qt + 1))
                ut = 2 + 2 * (u % UQ)   # first 512-tile of unit u in quarter
                lhsT = t_sb[ps, qg // 4, :, qq:qq + P]
                if qg == 0:
                    # absorb the DMA wait of the chunk this unit reaches into
                    need_ck = 4 * qt + (ut + 1) // CHUNK_T
                    if need_ck > absorbed_ck:
                        link("PE", nc.tensor.ldweights(
                            weights=t_sb[ps, ut + 1, :, N_TILE - 1:N_TILE],
                            tile_position=(32 * qt, 0)))
                        absorbed_ck = need_ck

                if kind != "R" and n_a % 4 == 0 and n_a >= 4:
                    # ACT absorbers: pull the DVE clock forward so s-slot
                    # reuse WARs are already observed. Read LAGGED outputs
                    # (3 units back) so ACT never stalls behind DVE's newest
                    # work (spoolD bufs=7 keeps coverage: copy at n+4 reuses
                    # slot n-3; fold1(n-3) <= fold2 of the absorbed entry).
                    tA1 = tA1s[(n_a // 4) % 2]
                    if dve_gm_cols:
                        g_old, c_old = dve_gm_cols[max(-len(dve_gm_cols), -3)]
                        link("ACT", nc.scalar.copy(tA1, g_old[0:1, c_old:c_old + 1]))
                        link("ACT", nc.scalar.copy(tA2, tA1))

                # PE absorber: standalone ldweights (no output; clobbered by
                # the next matmul's weight load) reading the output of the
                # consumer that released this pair's PSUM slot.
                if kind == "R":
                    dsrc = r_srcs[n_r - 2] if n_r >= 2 else t_sb[0:32, 0, 0, 0:1]
                else:
                    dsrc = a_srcs[n_a - 2] if n_a >= 2 else t_sb[0:32, 0, 0, 0:1]
                link("PE", nc.tensor.ldweights(weights=dsrc))

                pool_, tag_ = (ppoolR, "ppR") if kind == "R" else (ppoolA, "ppA")
                p = pool_.tile([P, PAIR_W], mybir.dt.float32, tag=tag_)
                for j in range(2):
                    link("PE", nc.tensor.matmul(
                        p[:, j * N_TILE:(j + 1) * N_TILE], lhsT,
                        t_sb[ps, ut + j], start=True, stop=True,
                        perf_mode=mybir.MatmulPerfMode.DoubleRow))

                if kind == "R":
                    gbase = dve_base
                    dve_base += OCT_W
                    link("DVE", nc.vector.tensor_reduce(
                        out=gm_tile[:, gbase:gbase + OCT_W],
                        in_=p.rearrange("p (g o) -> p o g", g=G),
                        axis=mybir.AxisListType.X,
                        op=mybir.AluOpType.max,
                    ))
                    r_srcs.append(gm_tile[0:D, gbase + OCT_W - 1:gbase + OCT_W])
                    dve_gm_cols.append((gm_tile, gbase + OCT_W - 1))
                    n_r += 1
                else:
                    s = spoolD.tile([P, PAIR_W], mybir.dt.bfloat16, tag="sD")
                    link("ACT", nc.scalar.copy(s, p))
                    a_srcs.append(s[0:D, 0:1])
                    gbase = dve_base
                    dve_base += 4 * OCT_W
                    link("DVE", nc.vector.tensor_max(
                        gm_tile[:, gbase:gbase + 4 * OCT_W],
                        s[:, 0:512], s[:, 512:1024]))
                    dve_gm_cols.append((gm_tile, gbase + 4 * OCT_W - 1))
                    n_a += 1

            # one output DMA per qg (single writer engine: DVE; 8 DMAs =
            # NUM_HWDGE_SEMS, so no hwdge sem reuse)
            nc.sync.dma_start(out=gm[qg * P:(qg + 1) * P, :], in_=gm_tile)
            prev_gms.append(gm_tile)

    # wait audit: walrus accepts at most one semaphore wait per instruction
    bad = []
    for blk in nc.m.functions[0].blocks:
        for inst in blk.instructions:
            si = inst.sync_info
            if si is None:
                continue
            nw = len(si.on_wait or [])
            if nw > 1:
                bad.append((type(inst).__name__, inst.name, nw))
    assert not bad, f"multi-wait instructions: {bad[:10]}"

    _NC_CACHE["nc"] = nc
    return nc


def _preprocess(table_keys):
    """Norm-sort, stripe across cores, member-major octet layout per unit.
    Octet o of unit u covers local norm ranks [8*(128u+o), ...+8); device
    column (within unit) = member*128 + o. gm holds 1 col/octet for R units
    and 2 cols/octet (member-parity pair maxima) for A1/A2 units; the host
    maxes the pair. Returns per-core fp8 tables plus octet-indexed candidate
    rows / min-norms and the octet -> gm column maps."""
    tk = np.ascontiguousarray(table_keys, dtype=np.float32)
    norms = np.einsum("ij,ij->i", tk, tk)
    order = np.argsort(norms, kind="stable")

    # octet id (u*128+o) -> up to 4 gm columns (host maxes them; R units
    # repeat their single column)
    oct2cols = np.zeros((4, N_OCT), dtype=np.int64)
    dve_base = 0
    for u, kind in enumerate(UNITS):
        o = np.arange(OCT_W)
        oid = u * OCT_W + o
        if kind == "R":
            for j in range(4):
                oct2cols[j, oid] = dve_base + o
            dve_base += OCT_W
        else:
            for j in range(4):
                oct2cols[j, oid] = dve_base + j * OCT_W + o
            dve_base += 4 * OCT_W

    tts = []
    cand_rows = np.full((CORES, N_OCT, G), -1, dtype=np.int64)
    gnorm_min = np.full((CORES, N_OCT), np.float32(1e9), dtype=np.float32)

    n_loc = C // CORES             # 62500 real rows per core
    # member-major permutation within a unit: rank r_in -> col (r_in%8)*128 + r_in//8
    r_in = np.arange(PAIR_W)
    col_of_rin = (r_in % G) * OCT_W + (r_in // G)

    for m in range(CORES):
        rows_m = order[m::CORES]
        Ts = np.zeros((N_COLS, D), dtype=np.float32)      # rank-indexed
        Ts[:n_loc] = tk[rows_m]
        nrm = np.full(N_COLS, np.float32(1e9), dtype=np.float32)
        nrm[:n_loc] = norms[rows_m]

        Tdev = np.zeros((C_LOC, D), dtype=np.float32)     # position-indexed
        for u in range(N_UNITS):
            base = u * PAIR_W
            Tdev[base + col_of_rin] = Ts[base + r_in]

        oid = np.arange(N_OCT)
        ranks = oid[:, None] * G + np.arange(G)[None, :]
        valid = ranks < n_loc
        ranks_c = np.minimum(ranks, n_loc - 1)
        cand_rows[m] = np.where(valid, rows_m[ranks_c], -1)
        gnorm_min[m] = np.where(valid[:, 0],
                                nrm[np.minimum(ranks[:, 0], N_COLS - 1)],
                                np.float32(1e9))

        tts.append(Tdev.T.astype(ml_dtypes.float8_e4m3))   # [64, C_LOC]

    return tts, cand_rows, gnorm_min, oct2cols


def kernel(keys, table_keys, table_values):
    q = np.ascontiguousarray(keys, dtype=np.float32)
    tk = np.ascontiguousarray(table_keys, dtype=np.float32)
    v = np.ascontiguousarray(table_values, dtype=np.float32)

    tts, cand_rows, gnorm_min, oct2cols = _preprocess(tk)
    q8 = q.T.astype(ml_dtypes.float8_e4m3)                 # [64, B]
    # per quarter: [64, (2 + 32)*512 cols] -> [32, NT_Q, 2, 512]; quarters
    # stacked on the partition axis -> [128, NT_Q, 2, 512]
    tqs = []
    for m in range(CORES):
        full = np.zeros((96, NT_Q, 2, N_TILE), dtype=ml_dtypes.float8_e4m3)
        for qt in range(NQUAD):
            u0 = qt * UQ
            nu = min(UQ, N_UNITS - u0)
            cols = np.concatenate(
                [q8, tts[m][:, u0 * PAIR_W:(u0 + nu) * PAIR_W]], axis=1)
            nt = 2 + 2 * nu
            packed = cols.reshape(2, 32, nt, N_TILE).transpose(1, 2, 0, 3)
            full[32 * qt:32 * (qt + 1), :nt] = packed
        tqs.append(full)

    nc = _build_nc()
    in_maps = [{"tq": tqs[m]} for m in range(CORES)]
    res = run_bass_kernel_spmd(nc, in_maps, core_ids=list(range(CORES)))
    gmax = np.stack([r["gm"].astype(np.float32) for r in res.results])  # [8, B, GM_W]

    # ---- host stage 2: rank octets by distance lower bound ----
    gmax = np.nan_to_num(gmax, nan=-1e9, posinf=-1e9, neginf=-1e9)
    gmax_oct = gmax[:, :, oct2cols[0]]
    for j in range(1, 4):
        gmax_oct = np.maximum(gmax_oct, gmax[:, :, oct2cols[j]])  # [8,B,N_OCT]
    invalid_g = gnorm_min >= np.float32(1e9)         # [8, N_OCT]
    gmax_oct = np.where(invalid_g[:, None, :], np.float32(-1e9), gmax_oct)
    lb = gnorm_min[:, None, :] - 2.0 * gmax_oct      # [8, B, N_OCT]
    lb = lb.transpose(1, 0, 2).reshape(B, CORES * N_OCT)
    top_g = np.argpartition(lb, N_GROUPS, axis=1)[:, :N_GROUPS]

    core_of = top_g // N_OCT
    g_of = top_g % N_OCT
    rows = cand_rows[core_of, g_of].reshape(B, N_GROUPS * G)
    invalid = rows < 0
    rows_safe = np.where(invalid, 0, rows)

    # ---- exact rescore with the reference's formula (f32) ----
    tc_ = tk[rows_safe]                               # [B, NCAND, D]
    qn = np.einsum("ij,ij->i", q, q)
    tn = np.einsum("ij,ij->i", tk, tk)[rows_safe]
    dots = np.einsum("bd,bkd->bk", q, tc_)
    d2 = qn[:, None] - 2.0 * dots + tn
    d2 = np.where(invalid, np.float32(np.inf), d2).astype(np.float32)

    top_k = np.argpartition(d2, K, axis=1)[:, :K]
    rows_k = np.take_along_axis(rows_safe, top_k, axis=1)

    # ---- reference tail: exact sq, inverse-distance weights ----
    nb = tk[rows_k]
    sq = np.sum((q[:, None, :] - nb) ** 2, axis=2, dtype=np.float32)
    w = np.float32(1.0) / (sq + np.float32(DELTA))
    w = w / np.sum(w, axis=1, keepdims=True)
    out = np.sum(w * v[rows_k], axis=1)
    return out.astype(np.float32)



# revision 25
# speedup vs baseline: 1.3119x; 1.3119x over previous
"""Distributed brute-force kNN (top-50 inverse-distance-weighted regression), v3.

Strategy (8 NeuronCores):
  - Table (500k x 64) norm-sorted and striped across 8 cores (62500 rows each,
    padded to 63488 = 62 units x 1024 cols). fp8e4m3 everywhere on device;
    dots via DoubleRow matmuls (K packed as [32 partitions x 2 k-tiles]).
  - Octet-major device layout: within a unit, device col = rank (= octet*8 +
    member). Per 128-query group, each unit's 1024 dots fold 8->4 per octet
    (members m maxed with m+4) and ship as fp8; the host maxes the 4
    surviving cols per octet. Two balanced fold paths:
      D (38/62): DVE tensor_tensor(max) STRAIGHT from PSUM (in0/in1 = the
        two member-halves, max_ap 512 => 658ns for 1024 f32 read) -> gmD.
      A (24/62): ACT evicts PSUM -> bf16 SBUF (1038ns), Pool (gpsimd)
        scalar_tensor_tensor(max) folds -> gmA (gpsimd cannot touch PSUM).
  - gm staging tiles hold a PAIR of qgs ([P, 2*(W+1)] with a pad col per
    half) so the 16 DMA procs (8 hwdge + 8 swdge) suffice: every DMA sits on
    a fresh proc and carries at most ONE wait (its writer RAW; walrus allows
    a single semaphore wait per instruction). hwdge: 4 input chunks + 3
    D-pair ships + the last D tail chunk; swdge: 3 A-pair ships + the qg6
    halves + qg7 tail chunks. The last qg ships in halves to cut the tail.
  - Host: rank octets by the distance lower bound min_norm - 2*gmax, take
    top N_GROUPS=256 octets, re-score the 8*N_GROUPS candidates exactly in
    f32, final top-50 + inverse-distance weights (reference formula).

Sync discipline: per-engine total order via nosync deps (link()); every
cross-engine hazard is carried by exactly one instruction wait. Absorber ops
pull engine clocks forward so Tile never emits a second wait: standalone PE
ldweights absorb PSUM-slot WARs and input-chunk RAWs; paired tiny ACT copies
absorb the s-slot WAR (Pool read) and the ACT self-clock; at pair-tile
recycling boundaries a PUMP (reads a cell the engine wrote last qg; one
self-sem wait pulling the engine's observed self-clock current) precedes the
tile allocation and a CATCHER (writes the pad col the recycled tile's ship
read) takes the ship-DMA WAR as its only wait.
"""

import numpy as np
import ml_dtypes

import concourse.bass as bass
import concourse.tile as tile
from concourse import mybir
from concourse.bass_utils import run_bass_kernel_spmd
from contextlib import ExitStack

# Problem geometry (hardcoded per spec).
B = 1024          # queries
D = 64            # feature dim
C = 500000        # table capacity
K = 50            # neighbours
DELTA = 1e-3

CORES = 8
N_TILE = 512      # matmul free dim (one PSUM bank)
PAIR_W = 2 * N_TILE
N_UNITS = 62      # 62 * 1024 = 63488 >= 62500 real rows per core
N_COLS = N_UNITS * PAIR_W
C_LOC = 63488     # table width in tq (= N_COLS exactly)
G = 8             # octet size
OCT_W = PAIR_W // G            # 128 octets per unit
N_OCT = N_UNITS * OCT_W        # 7936 octets per core
P = 128           # partition dim == query-group size
QG = B // P       # 8 query groups

N_GROUPS = 256    # candidate octets per query taken on host

# unit kinds: D = DVE tensor_reduce straight from PSUM (full octet fold to
# 128 cols; walrus allows just ONE PSUM operand per instruction, so the
# two-input PSUM folds are illegal); A = ACT evicts PSUM straight to fp8 gm
# cols (1024/unit; the host maxes the 8 members -- Pool/gpsimd has no
# working tensor ops in this toolchain, so there is no cheap fold stage).
# Counts balance DVE (1192ns/unit) vs ACT (1038ns/unit).
N_D = 29
N_A = 33


def _mk_units(n_d=N_D, n_a=N_A):
    # largest-remainder interleave for smooth engine load
    counts = {k: v for k, v in (("D", n_d), ("A", n_a)) if v > 0}
    acc = {k: 0.0 for k in counts}
    out = []
    for i in range(N_UNITS):
        for k in counts:
            acc[k] += counts[k] / N_UNITS
        k = max(acc, key=lambda kk: acc[kk])
        acc[k] -= 1.0
        out.append(k)
    i0 = out.index("D")
    out = out[i0:] + out[:i0]
    assert all(out.count(k) == counts.get(k, 0) for k in ("D", "A"))
    return out


UNITS = _mk_units()
DB_W = N_D * OCT_W + 1                  # D block + pad col (per qg half)
AB_W = N_A * PAIR_W + 1                 # A block + pad col (per qg half)
GM_W = DB_W + AB_W                      # DRAM gm row width
GM_DT = mybir.dt.float8e4
GM_NP_DT = ml_dtypes.float8_e4m3

# qg7 tail ship boundaries (A-unit index); the final chunk is smallest so
# the drain tail is short. The D block is small enough to ship whole.
A_TAIL_AT = (16, 28, N_A)

# quadrant layout: table third qt (21 units) lives on partitions
# [32qt, 32qt+32) with its own query replica (PE row-group bases 0/32/64;
# base 96 = broken HW quadrant).
UQ = 21
NQUAD = 3
NT_Q = 2 + 2 * UQ                  # 44 512-col tiles per third (2 = queries)

# input chunks (SP hwdge, procs 0-3): (third, tile_lo, tile_hi, first unit
# that needs it). Chunk 0 = queries + first 5 units of third 0 so compute
# starts early; the PE absorber for chunk k sits before unit need_u.
IN_CHUNKS = ((0, 0, 12, 0), (0, 12, NT_Q, 5), (1, 0, NT_Q, 21), (2, 0, NT_Q, 42))

_NC_CACHE = {}


def _build_nc():
    if "nc" in _NC_CACHE:
        return _NC_CACHE["nc"]
    # swdge descriptor carveout sized so the ring never wraps (no ring waits)
    nc = bass.Bass(dynamic_dma_scratch_size=24576)
    # queries FIRST so the first weight loads only need the first DMA chunk
    # k-tiles interleaved per 512-col tile so every AP's k-step is 512
    tq = nc.declare_dram_parameter("tq", [96, NT_Q, 2, N_TILE],
                                   mybir.dt.float8e4, isOutput=False)
    gm = nc.declare_dram_parameter("gm", [B, GM_W], GM_DT, isOutput=True)

    from concourse.bass import _add_dep_helper as dep

    # Split TileContext's exit drain (one wait per active proc) into
    # one-wait-per-drain instructions.
    from concourse.vector_clock import VectorClock, ScopedClock

    def _split_drain_and_barrier(self, tick_clock, wait_clock):
        gc = tick_clock.global_clock
        for proc in range(27):
            t = gc.peek_next(proc) - 1
            if t <= 0:
                continue
            d = self.nc.sync.drain()
            pc = VectorClock()
            pc.require_at_least(proc, t)
            wait_clock.add_sem_waits(d.ins, ScopedClock({None: pc}))
        self.nc.all_engine_barrier()
        assert self.sems is not None
        popped = self.nc._tile_sem_poison_stack.pop()
        assert popped is self._sem_poison
        self.nc.clear_and_free_semaphores(list(self.sems.allocated().values()))
        self.nc.all_engine_barrier()

    tile.TileContext._drain_and_barrier = _split_drain_and_barrier

    with ExitStack() as ctx:
        tc = ctx.enter_context(tile.TileContext(nc, pool_alloc_mode="stack"))
        singles = ctx.enter_context(tc.tile_pool(name="singles", bufs=1))
        # one PSUM pool per consumer engine: within a tag all releases come
        # from one engine in issue order, so slot reuse is deterministic
        ppoolD = ctx.enter_context(tc.tile_pool(name="ppoolD", bufs=2, space="PSUM"))
        ppoolA = ctx.enter_context(tc.tile_pool(name="ppoolA", bufs=2, space="PSUM"))
        gmpoolD = ctx.enter_context(tc.tile_pool(name="gmpoolD", bufs=2))
        gmpoolA = ctx.enter_context(tc.tile_pool(name="gmpoolA", bufs=2))
        apool = ctx.enter_context(tc.tile_pool(name="apool", bufs=2))

        # chunked table load on the SP hwdge ring (procs 0-3): SP has no
        # engine work, hwdge descriptor gen is off the compute engines, and
        # the swdge procs stay fresh for the A/tail ships.
        t_sb = singles.tile([96, NT_Q, 2, N_TILE], mybir.dt.float8e4)
        for (qt, tl, th, _nu) in IN_CHUNKS:
            ps = slice(32 * qt, 32 * (qt + 1))
            nc.sync.dma_start(out=t_sb[ps, tl:th], in_=tq[ps, tl:th])

        chain = {"PE": None, "DVE": None, "ACT": None, "POOL": None}

        def link(key, inst):
            # total order per engine queue (add_dep_helper(a, b) == a waits b)
            if chain[key] is not None:
                dep(inst.ins, chain[key].ins, sync=False, reason=f"{key} order")
            chain[key] = inst
            return inst

        dscr = apool.tile([1, 1], GM_DT, tag="dscr")    # DVE pump sink
        ascr = apool.tile([1, 1], GM_DT, tag="ascr")    # ACT pump sink
        # catcher sources, written ONCE at qg0: later reads are ancient
        # same-engine RAWs already covered by the pump's self-clock
        dzero = apool.tile([1, 1], GM_DT, tag="dzero")
        azero4 = apool.tile([1, 4], GM_DT, tag="azero")
        azero = azero4[0:1, 0:1]
        link("DVE", nc.vector.memset(dzero, 0.0))
        link("ACT", nc.scalar.memzero(azero4))

        d_srcs = []            # gm cell AP per D-unit (PE absorber source)
        a_srcs = []            # gm cell AP per A-unit (PE absorber source)
        nd = 0                 # global D-unit counter
        na = 0                 # global A-unit counter
        gmD = gmA = None

        ck_i = 0
        for qg in range(QG):
            qq = (qg % 4) * P
            half = qg % 2
            dh = half * DB_W            # col base of this qg's half in gmD
            ah = half * AB_W            # col base of this qg's half in gmA
            if half == 0:
                if qg >= 4:
                    # pair-tile recycling: PUMP (reads the previous pair
                    # tile's odd-half cell -- recent self-RAW pulling the
                    # engine's self-clock to the present), then allocate, then
                    # CATCHER (takes the recycled tile's pair-ship WAR via the
                    # pad col -- its single wait).
                    link("DVE", nc.vector.tensor_copy(
                        dscr, gmD[0:1, DB_W:DB_W + 1]))
                    link("ACT", nc.scalar.copy(
                        ascr, gmA[0:1, AB_W:AB_W + 1]))
                    gmD = gmpoolD.tile([P, 2 * DB_W], GM_DT, tag="gmD")
                    link("DVE", nc.vector.tensor_copy(
                        gmD[0:1, DB_W - 1:DB_W], dzero))
                    gmA = gmpoolA.tile([P, 2 * AB_W], GM_DT, tag="gmA")
                    link("ACT", nc.scalar.copy(
                        gmA[0:1, AB_W - 1:AB_W], azero))
                else:
                    gmD = gmpoolD.tile([P, 2 * DB_W], GM_DT, tag="gmD")
                    gmA = gmpoolA.tile([P, 2 * AB_W], GM_DT, tag="gmA")

            di = 0             # D-units completed within this qg
            ai = 0             # A-units completed within this qg
            a_tail_i = 0
            for u, kind in enumerate(UNITS):
                qt = u // UQ            # third (partition base 32*qt)
                ps = slice(32 * qt, 32 * (qt + 1))
                ut = 2 + 2 * (u % UQ)   # first 512-tile of unit u in third
                lhsT = t_sb[ps, qg // 4, :, qq:qq + P]
                if qg == 0 and ck_i < len(IN_CHUNKS) and u == IN_CHUNKS[ck_i][3]:
                    # absorb this chunk's DMA-complete wait into a ldweights
                    cqt, ctl, cth, _nu = IN_CHUNKS[ck_i]
                    cps = slice(32 * cqt, 32 * (cqt + 1))
                    link("PE", nc.tensor.ldweights(
                        weights=t_sb[cps, cth - 1, :, N_TILE - 1:N_TILE],
                        tile_position=(32 * cqt, 0)))
                    ck_i += 1

                # PE absorber: standalone ldweights (no output; clobbered by
                # the next matmul's weight load) reading the output of the
                # consumer that released this unit's PSUM slot.
                if kind == "D":
                    dsrc = d_srcs[nd - 2] if nd >= 2 else t_sb[0:32, 0, 0, 0:1]
                else:
                    dsrc = a_srcs[na - 2] if na >= 2 else t_sb[0:32, 0, 0, 0:1]
                link("PE", nc.tensor.ldweights(weights=dsrc))

                pool_, tag_ = (ppoolD, "ppD") if kind == "D" else (ppoolA, "ppA")
                p = pool_.tile([P, PAIR_W], mybir.dt.float32, tag=tag_)
                for j in range(2):
                    link("PE", nc.tensor.matmul(
                        p[:, j * N_TILE:(j + 1) * N_TILE], lhsT,
                        t_sb[ps, ut + j], start=True, stop=True,
                        perf_mode=mybir.MatmulPerfMode.DoubleRow))

                pv = p.rearrange("p (o m) -> p o m", m=G)
                if kind == "D":
                    gbase = dh + di * OCT_W
                    link("DVE", nc.vector.tensor_reduce(
                        out=gmD[:, gbase:gbase + OCT_W], in_=pv,
                        axis=mybir.AxisListType.X, op=mybir.AluOpType.max))
                    d_srcs.append(gmD[0:D, gbase:gbase + 1])
                    nd += 1
                    di += 1
                    if qg == QG - 1 and di == N_D:
                        # qg7 D block ships whole on a swdge proc
                        link("POOL", nc.gpsimd.dma_start(
                            out=gm[qg * P:(qg + 1) * P, 0:DB_W],
                            in_=gmD[:, dh:dh + DB_W]))
                else:
                    gbase = ah + ai * PAIR_W
                    link("ACT", nc.scalar.copy(
                        gmA[:, gbase:gbase + PAIR_W], p))
                    a_srcs.append(gmA[0:D, gbase:gbase + 1])
                    na += 1
                    ai += 1
                    if qg == QG - 1 and ai == A_TAIL_AT[a_tail_i]:
                        lo = (A_TAIL_AT[a_tail_i - 1] if a_tail_i else 0) * PAIR_W
                        hi = ai * PAIR_W if a_tail_i < len(A_TAIL_AT) - 1 else AB_W
                        link("POOL", nc.gpsimd.dma_start(
                            out=gm[qg * P:(qg + 1) * P, DB_W + lo:DB_W + hi],
                            in_=gmA[:, ah + lo:ah + hi]))
                        a_tail_i += 1

            if qg % 2 == 1 and qg < QG - 1:
                # pair ships (two qgs' rows in one DMA): D via SP hwdge, A via
                # the Pool swdge queue; one RAW wait each (single writer)
                rows = gm[(qg - 1) * P:(qg + 1) * P]
                nc.sync.dma_start(
                    out=rows[:, 0:DB_W].rearrange("(h p) c -> p h c", h=2),
                    in_=gmD.rearrange("p (h c) -> p h c", h=2))
                link("POOL", nc.gpsimd.dma_start(
                    out=rows[:, DB_W:GM_W].rearrange("(h p) c -> p h c", h=2),
                    in_=gmA.rearrange("p (h c) -> p h c", h=2)))
            elif qg == QG - 2:
                # qg6: ship this half now (qg7 ships at its end); D on the
                # last hwdge proc, A on a swdge proc
                nc.sync.dma_start(
                    out=gm[qg * P:(qg + 1) * P, 0:DB_W], in_=gmD[:, 0:DB_W])
                link("POOL", nc.gpsimd.dma_start(
                    out=gm[qg * P:(qg + 1) * P, DB_W:GM_W], in_=gmA[:, 0:AB_W]))

    # wait audit: walrus accepts at most one semaphore wait per instruction
    bad = []
    for blk in nc.m.functions[0].blocks:
        for inst in blk.instructions:
            si = inst.sync_info
            if si is None:
                continue
            nw = len(si.on_wait or [])
            if nw > 1:
                bad.append((type(inst).__name__, inst.name, nw))
    assert not bad, f"multi-wait instructions: {bad[:10]}"

    _NC_CACHE["nc"] = nc
    return nc


def _preprocess(table_keys):
    """Norm-sort, stripe across cores; octet-major device layout (device col
    within a unit = local rank, octet o covers ranks [8*(128u+o), ...+8)).
    gm holds 4 cols per octet (member-pair maxima from fold1); the host maxes
    them. Returns per-core fp8 tables plus octet-indexed candidate rows /
    min-norms and the octet -> gm column map."""
    tk = np.ascontiguousarray(table_keys, dtype=np.float32)
    norms = np.einsum("ij,ij->i", tk, tk)
    order = np.argsort(norms, kind="stable")

    # gm col base per unit: D block cols [0, N_D*OCT_W), pad, A block
    colbase = np.zeros(N_UNITS, dtype=np.int64)
    kinds = []
    dnext, anext = 0, DB_W
    for u, kind in enumerate(UNITS):
        kinds.append(kind)
        if kind == "D":
            colbase[u] = dnext
            dnext += OCT_W
        else:
            colbase[u] = anext
            anext += PAIR_W

    # octet id (u*128+o) -> its gm columns: D units have one col per octet
    # (repeated 8x), A units have the 8 raw member cols (host maxes them)
    oct2cols = np.zeros((8, N_OCT), dtype=np.int64)
    o = np.arange(OCT_W)
    for u in range(N_UNITS):
        oid = u * OCT_W + o
        for j in range(8):
            if kinds[u] == "D":
                oct2cols[j, oid] = colbase[u] + o
            else:
                oct2cols[j, oid] = colbase[u] + 8 * o + j

    tts = []
    cand_rows = np.full((CORES, N_OCT, G), -1, dtype=np.int64)
    gnorm_min = np.full((CORES, N_OCT), np.float32(1e9), dtype=np.float32)

    n_loc = C // CORES             # 62500 real rows per core

    for m in range(CORES):
        rows_m = order[m::CORES]
        Tdev = np.zeros((N_COLS, D), dtype=np.float32)   # rank == device col
        Tdev[:n_loc] = tk[rows_m]
        nrm = np.full(N_COLS, np.float32(1e9), dtype=np.float32)
        nrm[:n_loc] = norms[rows_m]

        oid = np.arange(N_OCT)
        ranks = oid[:, None] * G + np.arange(G)[None, :]
        valid = ranks < n_loc
        ranks_c = np.minimum(ranks, n_loc - 1)
        cand_rows[m] = np.where(valid, rows_m[ranks_c], -1)
        gnorm_min[m] = np.where(valid[:, 0], nrm[ranks[:, 0]], np.float32(1e9))

        tts.append(Tdev.T.astype(ml_dtypes.float8_e4m3))   # [64, C_LOC]

    return tts, cand_rows, gnorm_min, oct2cols


def kernel(keys, table_keys, table_values):
    q = np.ascontiguousarray(keys, dtype=np.float32)
    tk = np.ascontiguousarray(table_keys, dtype=np.float32)
    v = np.ascontiguousarray(table_values, dtype=np.float32)

    tts, cand_rows, gnorm_min, oct2cols = _preprocess(tk)
    q8 = q.T.astype(ml_dtypes.float8_e4m3)                 # [64, B]
    # per third: [64, (2 + 42)*512 cols] -> [32, NT_Q, 2, 512]; thirds
    # stacked on the partition axis -> [96, NT_Q, 2, 512]
    tqs = []
    for m in range(CORES):
        full = np.zeros((96, NT_Q, 2, N_TILE), dtype=ml_dtypes.float8_e4m3)
        for qt in range(NQUAD):
            u0 = qt * UQ
            nu = min(UQ, N_UNITS - u0)
            cols = np.concatenate(
                [q8, tts[m][:, u0 * PAIR_W:(u0 + nu) * PAIR_W]], axis=1)
            nt = 2 + 2 * nu
            packed = cols.reshape(2, 32, nt, N_TILE).transpose(1, 2, 0, 3)
            full[32 * qt:32 * (qt + 1), :nt] = packed
        tqs.append(full)

    nc = _build_nc()
    in_maps = [{"tq": tqs[m]} for m in range(CORES)]
    res = run_bass_kernel_spmd(nc, in_maps, core_ids=list(range(CORES)))
    gmax = np.stack([np.asarray(r["gm"]).astype(np.float32)
                     for r in res.results])              # [8, B, GM_W]

    # ---- host stage 2: rank octets by distance lower bound ----
    gmax = np.nan_to_num(gmax, nan=-1e9, posinf=-1e9, neginf=-1e9)
    gmax_oct = gmax[:, :, oct2cols[0]]
    for j in range(1, 8):
        gmax_oct = np.maximum(gmax_oct, gmax[:, :, oct2cols[j]])  # [8,B,N_OCT]
    invalid_g = gnorm_min >= np.float32(1e9)         # [8, N_OCT]
    gmax_oct = np.where(invalid_g[:, None, :], np.float32(-1e9), gmax_oct)
    lb = gnorm_min[:, None, :] - 2.0 * gmax_oct      # [8, B, N_OCT]
    lb = lb.transpose(1, 0, 2).reshape(B, CORES * N_OCT)
    top_g = np.argpartition(lb, N_GROUPS, axis=1)[:, :N_GROUPS]

    core_of = top_g // N_OCT
    g_of = top_g % N_OCT
    rows = cand_rows[core_of, g_of].reshape(B, N_GROUPS * G)
    invalid = rows < 0
    rows_safe = np.where(invalid, 0, rows)

    # ---- exact rescore with the reference's formula (f32) ----
    tc_ = tk[rows_safe]                               # [B, NCAND, D]
    qn = np.einsum("ij,ij->i", q, q)
    tn = np.einsum("ij,ij->i", tk, tk)[rows_safe]
    dots = np.einsum("bd,bkd->bk", q, tc_)
    d2 = qn[:, None] - 2.0 * dots + tn
    d2 = np.where(invalid, np.float32(np.inf), d2).astype(np.float32)

    top_k = np.argpartition(d2, K, axis=1)[:, :K]
    rows_k = np.take_along_axis(rows_safe, top_k, axis=1)

    # ---- reference tail: exact sq, inverse-distance weights ----
    nb = tk[rows_k]
    sq = np.sum((q[:, None, :] - nb) ** 2, axis=2, dtype=np.float32)
    w = np.float32(1.0) / (sq + np.float32(DELTA))
    w = w / np.sum(w, axis=1, keepdims=True)
    out = np.sum(w * v[rows_k], axis=1)
    return out.astype(np.float32)


# revision 27
# speedup vs baseline: 1.3235x; 1.0088x over previous
"""Distributed brute-force kNN (top-50 inverse-distance-weighted regression), v4.

Strategy (8 NeuronCores):
  - Table (500k x 64) norm-sorted and striped across 8 cores (62500 rows each,
    padded to 63488 = 62 units x 1024 cols). fp8e4m3 everywhere on device;
    dots via DoubleRow matmuls (K packed as [32 partitions x 2 k-tiles]).
  - Octet-major device layout: within a unit, device col = rank (= octet*8 +
    member). Per 128-query group the 62 units' PSUM dots drain through the
    only two engines that can legally read PSUM (walrus allows ONE PSUM
    operand per instruction; Pool/DMA cannot touch PSUM at all, and Pool has
    no working tensor ops in this toolchain):
      D (29/62): DVE tensor_reduce straight from PSUM -- full octet max to
        128 fp8 gm cols per unit (1192ns).
      A (33/62): ACT evicts PSUM straight to 1024 raw fp8 gm cols per unit
        (1038ns); the host maxes the 8 members per octet.
    Both engines run ~94% busy; this split balances them.
  - gm staging tiles hold a PAIR of qgs ([P, 2*(W+1)] with a pad col per
    half) so the 16 DMA procs (8 hwdge + 8 swdge) suffice: every DMA sits on
    a fresh proc and carries at most ONE wait (its writer RAW; walrus allows
    a single semaphore wait per instruction). hwdge: 4 input chunks + 3
    D-pair ships + the qg6 D half; swdge: 3 A-pair ships + the qg6 A half +
    qg7 tail chunks (3 for A, smallest last, + the whole D block).
  - Host: rank octets by the distance lower bound min_norm - 2*gmax, take
    top N_GROUPS=256 octets, re-score the 8*N_GROUPS candidates exactly in
    f32, final top-50 + inverse-distance weights (reference formula).

Sync discipline: per-engine total order via nosync deps (link()); every
cross-engine hazard is carried by exactly one instruction wait. Absorber ops
pull engine clocks forward so Tile never emits a second wait: standalone PE
ldweights absorb PSUM-slot WARs and input-chunk RAWs; at pair-tile recycling
boundaries a PUMP (reads a cell the engine wrote last qg; one self-sem wait
pulling the engine's observed self-clock current) precedes the tile
allocation and a CATCHER (writes the pad col the recycled tile's ship read,
sourced from a scratch cell written once at qg0) takes the ship-DMA WAR as
its only wait.
"""

import numpy as np
import ml_dtypes

import concourse.bass as bass
import concourse.tile as tile
from concourse import mybir
from concourse.bass_utils import run_bass_kernel_spmd
from contextlib import ExitStack

# Problem geometry (hardcoded per spec).
B = 1024          # queries
D = 64            # feature dim
C = 500000        # table capacity
K = 50            # neighbours
DELTA = 1e-3

CORES = 8
N_TILE = 512      # matmul free dim (one PSUM bank)
PAIR_W = 2 * N_TILE
N_UNITS = 62      # 62 * 1024 = 63488 >= 62500 real rows per core
N_COLS = N_UNITS * PAIR_W
C_LOC = 63488     # table width in tq (= N_COLS exactly)
G = 8             # octet size
OCT_W = PAIR_W // G            # 128 octets per unit
N_OCT = N_UNITS * OCT_W        # 7936 octets per core
P = 128           # partition dim == query-group size
QG = B // P       # 8 query groups

N_GROUPS = 256    # candidate octets per query taken on host

# unit kinds: D = DVE tensor_reduce straight from PSUM (full octet fold to
# 128 cols; walrus allows just ONE PSUM operand per instruction, so the
# two-input PSUM folds are illegal); A = ACT evicts PSUM straight to fp8 gm
# cols (1024/unit; the host maxes the 8 members -- Pool/gpsimd has no
# working tensor ops in this toolchain, so there is no cheap fold stage).
# Counts balance DVE (1192ns/unit) vs ACT (1038ns/unit).
N_D = 29
N_A = 33


def _mk_units(n_d=N_D, n_a=N_A):
    # largest-remainder interleave for smooth engine load
    counts = {k: v for k, v in (("D", n_d), ("A", n_a)) if v > 0}
    acc = {k: 0.0 for k in counts}
    out = []
    for i in range(N_UNITS):
        for k in counts:
            acc[k] += counts[k] / N_UNITS
        k = max(acc, key=lambda kk: acc[kk])
        acc[k] -= 1.0
        out.append(k)
    i0 = out.index("D")
    out = out[i0:] + out[:i0]
    assert all(out.count(k) == counts.get(k, 0) for k in ("D", "A"))
    return out


UNITS = _mk_units()
# unit 61 must be a D unit (its half-width saving lands on the binding DVE)
if UNITS[61] != "D":
    _j = max(i for i, k in enumerate(UNITS) if k == "D")
    UNITS[61], UNITS[_j] = UNITS[_j], UNITS[61]
DB_W = (N_D - 1) * OCT_W + OCT_W // 2 + 1   # D block + pad (u61 is half)
AB_W = N_A * PAIR_W + 1                 # A block + pad col (per qg half)
GM_W = DB_W + AB_W                      # DRAM gm row width
GM_DT = mybir.dt.float8e4
GM_NP_DT = ml_dtypes.float8_e4m3

# qg7 tail ship boundaries (A-unit index); the final chunk is smallest so
# the drain tail is short. The D block is small enough to ship whole.
A_TAIL_AT = (16, 28, N_A)

# quadrant layout: table third qt (21 units) lives on partitions
# [32qt, 32qt+32) with its own query replica (PE row-group bases 0/32/64;
# base 96 = broken HW quadrant).
UQ = 21
NQUAD = 3
NT_Q = 2 + 2 * UQ                  # 44 512-col tiles per third (2 = queries)

# input chunks (SP hwdge, procs 0-3): (part_lo, part_hi, tile_lo, tile_hi,
# first unit that needs it). Chunk 0 = queries + unit 0 only so compute
# starts as early as possible; thirds 1+2 merge into one 64-partition chunk
# to stay within 4 hwdge procs. The PE absorber for chunk k sits before
# unit need_u.
IN_CHUNKS = ((0, 32, 0, 4, 0), (0, 32, 4, 12, 1), (0, 32, 12, NT_Q, 5),
             (32, 96, 0, NT_Q, 21))

# unit 61 covers ranks [62464, 63488) but only 36 rows (< 62500) are real:
# its whole second 512-tile is padding, so it runs one matmul and a
# half-width tensor_reduce (658ns instead of 1192 on the binding DVE).
HALF_U = 61
HALF_OCT = OCT_W // 2

_NC_CACHE = {}


def _build_nc():
    if "nc" in _NC_CACHE:
        return _NC_CACHE["nc"]
    # swdge descriptor carveout sized so the ring never wraps (no ring waits)
    nc = bass.Bass(dynamic_dma_scratch_size=24576)
    # queries FIRST so the first weight loads only need the first DMA chunk
    # k-tiles interleaved per 512-col tile so every AP's k-step is 512
    tq = nc.declare_dram_parameter("tq", [96, NT_Q, 2, N_TILE],
                                   mybir.dt.float8e4, isOutput=False)
    gm = nc.declare_dram_parameter("gm", [B, GM_W], GM_DT, isOutput=True)

    from concourse.bass import _add_dep_helper as dep

    # Split TileContext's exit drain (one wait per active proc) into
    # one-wait-per-drain instructions.
    from concourse.vector_clock import VectorClock, ScopedClock

    def _split_drain_and_barrier(self, tick_clock, wait_clock):
        gc = tick_clock.global_clock
        for proc in range(27):
            t = gc.peek_next(proc) - 1
            if t <= 0:
                continue
            d = self.nc.sync.drain()
            pc = VectorClock()
            pc.require_at_least(proc, t)
            wait_clock.add_sem_waits(d.ins, ScopedClock({None: pc}))
        self.nc.all_engine_barrier()
        assert self.sems is not None
        popped = self.nc._tile_sem_poison_stack.pop()
        assert popped is self._sem_poison
        self.nc.clear_and_free_semaphores(list(self.sems.allocated().values()))
        self.nc.all_engine_barrier()

    tile.TileContext._drain_and_barrier = _split_drain_and_barrier

    with ExitStack() as ctx:
        tc = ctx.enter_context(tile.TileContext(nc, pool_alloc_mode="stack"))
        singles = ctx.enter_context(tc.tile_pool(name="singles", bufs=1))
        # one PSUM pool per consumer engine: within a tag all releases come
        # from one engine in issue order, so slot reuse is deterministic
        ppoolD = ctx.enter_context(tc.tile_pool(name="ppoolD", bufs=2, space="PSUM"))
        ppoolA = ctx.enter_context(tc.tile_pool(name="ppoolA", bufs=2, space="PSUM"))
        gmpoolD = ctx.enter_context(tc.tile_pool(name="gmpoolD", bufs=2))
        gmpoolA = ctx.enter_context(tc.tile_pool(name="gmpoolA", bufs=2))
        apool = ctx.enter_context(tc.tile_pool(name="apool", bufs=2))

        # chunked table load on the SP hwdge ring (procs 0-3): SP has no
        # engine work, hwdge descriptor gen is off the compute engines, and
        # the swdge procs stay fresh for the A/tail ships.
        t_sb = singles.tile([96, NT_Q, 2, N_TILE], mybir.dt.float8e4)
        for (pl, ph, tl, th, _nu) in IN_CHUNKS:
            nc.sync.dma_start(out=t_sb[pl:ph, tl:th], in_=tq[pl:ph, tl:th])

        chain = {"PE": None, "DVE": None, "ACT": None, "POOL": None}

        def link(key, inst):
            # total order per engine queue (add_dep_helper(a, b) == a waits b)
            if chain[key] is not None:
                dep(inst.ins, chain[key].ins, sync=False, reason=f"{key} order")
            chain[key] = inst
            return inst

        dscr = apool.tile([1, 1], GM_DT, tag="dscr")    # DVE pump sink
        ascr = apool.tile([1, 1], GM_DT, tag="ascr")    # ACT pump sink
        # catcher sources, written ONCE at qg0: later reads are ancient
        # same-engine RAWs already covered by the pump's self-clock
        dzero = apool.tile([1, 1], GM_DT, tag="dzero")
        azero4 = apool.tile([1, 4], GM_DT, tag="azero")
        azero = azero4[0:1, 0:1]
        link("DVE", nc.vector.memset(dzero, 0.0))
        link("ACT", nc.scalar.memzero(azero4))

        d_srcs = []            # gm cell AP per D-unit (PE absorber source)
        a_srcs = []            # gm cell AP per A-unit (PE absorber source)
        nd = 0                 # global D-unit counter
        na = 0                 # global A-unit counter
        gmD = gmA = None

        ck_i = 0
        for qg in range(QG):
            qq = (qg % 4) * P
            half = qg % 2
            dh = half * DB_W            # col base of this qg's half in gmD
            ah = half * AB_W            # col base of this qg's half in gmA
            if half == 0:
                if qg >= 4:
                    # pair-tile recycling: PUMP (reads the previous pair
                    # tile's odd-half cell -- recent self-RAW pulling the
                    # engine's self-clock to the present), then allocate, then
                    # CATCHER (takes the recycled tile's pair-ship WAR via the
                    # pad col -- its single wait).
                    link("DVE", nc.vector.tensor_copy(
                        dscr, gmD[0:1, DB_W:DB_W + 1]))
                    link("ACT", nc.scalar.copy(
                        ascr, gmA[0:1, AB_W:AB_W + 1]))
                    gmD = gmpoolD.tile([P, 2 * DB_W], GM_DT, tag="gmD")
                    link("DVE", nc.vector.tensor_copy(
                        gmD[0:1, DB_W - 1:DB_W], dzero))
                    gmA = gmpoolA.tile([P, 2 * AB_W], GM_DT, tag="gmA")
                    link("ACT", nc.scalar.copy(
                        gmA[0:1, AB_W - 1:AB_W], azero))
                else:
                    gmD = gmpoolD.tile([P, 2 * DB_W], GM_DT, tag="gmD")
                    gmA = gmpoolA.tile([P, 2 * AB_W], GM_DT, tag="gmA")

            di = 0             # D-units completed within this qg
            ai = 0             # A-units completed within this qg
            a_tail_i = 0
            for u, kind in enumerate(UNITS):
                qt = u // UQ            # third (partition base 32*qt)
                ps = slice(32 * qt, 32 * (qt + 1))
                ut = 2 + 2 * (u % UQ)   # first 512-tile of unit u in third
                lhsT = t_sb[ps, qg // 4, :, qq:qq + P]
                if qg == 0 and ck_i < len(IN_CHUNKS) and u == IN_CHUNKS[ck_i][4]:
                    # absorb this chunk's DMA-complete wait into a ldweights
                    cpl, cph, ctl, cth, _nu = IN_CHUNKS[ck_i]
                    link("PE", nc.tensor.ldweights(
                        weights=t_sb[cph - 32:cph, cth - 1, :, N_TILE - 1:N_TILE],
                        tile_position=(cph - 32, 0)))
                    ck_i += 1

                # PE absorber: standalone ldweights (no output; clobbered by
                # the next matmul's weight load) reading the output of the
                # consumer that released this unit's PSUM slot.
                if kind == "D":
                    dsrc = d_srcs[nd - 2] if nd >= 2 else t_sb[0:32, 0, 0, 0:1]
                else:
                    dsrc = a_srcs[na - 2] if na >= 2 else t_sb[0:32, 0, 0, 0:1]
                link("PE", nc.tensor.ldweights(weights=dsrc))

                pool_, tag_ = (ppoolD, "ppD") if kind == "D" else (ppoolA, "ppA")
                p = pool_.tile([P, PAIR_W], mybir.dt.float32, tag=tag_)
                nmm = 1 if u == HALF_U else 2
                for j in range(nmm):
                    link("PE", nc.tensor.matmul(
                        p[:, j * N_TILE:(j + 1) * N_TILE], lhsT,
                        t_sb[ps, ut + j], start=True, stop=True,
                        perf_mode=mybir.MatmulPerfMode.DoubleRow))

                pv = p.rearrange("p (o m) -> p o m", m=G)
                if kind == "D":
                    w = HALF_OCT if u == HALF_U else OCT_W
                    gbase = dh + di * OCT_W
                    link("DVE", nc.vector.tensor_reduce(
                        out=gmD[:, gbase:gbase + w], in_=pv[:, 0:w, :],
                        axis=mybir.AxisListType.X, op=mybir.AluOpType.max))
                    d_srcs.append(gmD[0:D, gbase:gbase + 1])
                    nd += 1
                    di += 1
                    if qg == QG - 1 and di == N_D:
                        # qg7 D block ships whole on a swdge proc
                        link("POOL", nc.gpsimd.dma_start(
                            out=gm[qg * P:(qg + 1) * P, 0:DB_W],
                            in_=gmD[:, dh:dh + DB_W]))
                else:
                    gbase = ah + ai * PAIR_W
                    link("ACT", nc.scalar.copy(
                        gmA[:, gbase:gbase + PAIR_W], p))
                    a_srcs.append(gmA[0:D, gbase:gbase + 1])
                    na += 1
                    ai += 1
                    if qg == QG - 1 and ai == A_TAIL_AT[a_tail_i]:
                        lo = (A_TAIL_AT[a_tail_i - 1] if a_tail_i else 0) * PAIR_W
                        hi = ai * PAIR_W if a_tail_i < len(A_TAIL_AT) - 1 else AB_W
                        link("POOL", nc.gpsimd.dma_start(
                            out=gm[qg * P:(qg + 1) * P, DB_W + lo:DB_W + hi],
                            in_=gmA[:, ah + lo:ah + hi]))
                        a_tail_i += 1

            if qg % 2 == 1 and qg < QG - 1:
                # pair ships (two qgs' rows in one DMA): D via SP hwdge, A via
                # the Pool swdge queue; one RAW wait each (single writer)
                rows = gm[(qg - 1) * P:(qg + 1) * P]
                nc.sync.dma_start(
                    out=rows[:, 0:DB_W].rearrange("(h p) c -> p h c", h=2),
                    in_=gmD.rearrange("p (h c) -> p h c", h=2))
                link("POOL", nc.gpsimd.dma_start(
                    out=rows[:, DB_W:GM_W].rearrange("(h p) c -> p h c", h=2),
                    in_=gmA.rearrange("p (h c) -> p h c", h=2)))
            elif qg == QG - 2:
                # qg6: ship this half now (qg7 ships at its end); D on the
                # last hwdge proc, A on a swdge proc
                nc.sync.dma_start(
                    out=gm[qg * P:(qg + 1) * P, 0:DB_W], in_=gmD[:, 0:DB_W])
                link("POOL", nc.gpsimd.dma_start(
                    out=gm[qg * P:(qg + 1) * P, DB_W:GM_W], in_=gmA[:, 0:AB_W]))

    # wait audit: walrus accepts at most one semaphore wait per instruction
    bad = []
    for blk in nc.m.functions[0].blocks:
        for inst in blk.instructions:
            si = inst.sync_info
            if si is None:
                continue
            nw = len(si.on_wait or [])
            if nw > 1:
                bad.append((type(inst).__name__, inst.name, nw))
    assert not bad, f"multi-wait instructions: {bad[:10]}"

    _NC_CACHE["nc"] = nc
    return nc


def _preprocess(table_keys):
    """Norm-sort, stripe across cores; octet-major device layout (device col
    within a unit = local rank, octet o covers ranks [8*(128u+o), ...+8)).
    gm holds 4 cols per octet (member-pair maxima from fold1); the host maxes
    them. Returns per-core fp8 tables plus octet-indexed candidate rows /
    min-norms and the octet -> gm column map."""
    tk = np.ascontiguousarray(table_keys, dtype=np.float32)
    norms = np.einsum("ij,ij->i", tk, tk)
    order = np.argsort(norms, kind="stable")

    # gm col base per unit: D block cols [0, N_D*OCT_W), pad, A block
    colbase = np.zeros(N_UNITS, dtype=np.int64)
    kinds = []
    dnext, anext = 0, DB_W
    for u, kind in enumerate(UNITS):
        kinds.append(kind)
        if kind == "D":
            colbase[u] = dnext
            dnext += OCT_W
        else:
            colbase[u] = anext
            anext += PAIR_W

    # octet id (u*128+o) -> its gm columns: D units have one col per octet
    # (repeated 8x), A units have the 8 raw member cols (host maxes them)
    oct2cols = np.zeros((8, N_OCT), dtype=np.int64)
    o = np.arange(OCT_W)
    for u in range(N_UNITS):
        oid = u * OCT_W + o
        for j in range(8):
            if kinds[u] == "D":
                # u61's octets beyond its half width are padding (rank >=
                # 62500, masked invalid by gnorm_min); clamp their col
                oc = np.minimum(o, OCT_W // 2 - 1) if u == 61 else o
                oct2cols[j, oid] = colbase[u] + oc
            else:
                oct2cols[j, oid] = colbase[u] + 8 * o + j

    tts = []
    cand_rows = np.full((CORES, N_OCT, G), -1, dtype=np.int64)
    gnorm_min = np.full((CORES, N_OCT), np.float32(1e9), dtype=np.float32)

    n_loc = C // CORES             # 62500 real rows per core

    for m in range(CORES):
        rows_m = order[m::CORES]
        Tdev = np.zeros((N_COLS, D), dtype=np.float32)   # rank == device col
        Tdev[:n_loc] = tk[rows_m]
        nrm = np.full(N_COLS, np.float32(1e9), dtype=np.float32)
        nrm[:n_loc] = norms[rows_m]

        oid = np.arange(N_OCT)
        ranks = oid[:, None] * G + np.arange(G)[None, :]
        valid = ranks < n_loc
        ranks_c = np.minimum(ranks, n_loc - 1)
        cand_rows[m] = np.where(valid, rows_m[ranks_c], -1)
        gnorm_min[m] = np.where(valid[:, 0], nrm[ranks[:, 0]], np.float32(1e9))

        tts.append(Tdev.T.astype(ml_dtypes.float8_e4m3))   # [64, C_LOC]

    return tts, cand_rows, gnorm_min, oct2cols


def kernel(keys, table_keys, table_values):
    q = np.ascontiguousarray(keys, dtype=np.float32)
    tk = np.ascontiguousarray(table_keys, dtype=np.float32)
    v = np.ascontiguousarray(table_values, dtype=np.float32)

    tts, cand_rows, gnorm_min, oct2cols = _preprocess(tk)
    q8 = q.T.astype(ml_dtypes.float8_e4m3)                 # [64, B]
    # per third: [64, (2 + 42)*512 cols] -> [32, NT_Q, 2, 512]; thirds
    # stacked on the partition axis -> [96, NT_Q, 2, 512]
    tqs = []
    for m in range(CORES):
        full = np.zeros((96, NT_Q, 2, N_TILE), dtype=ml_dtypes.float8_e4m3)
        for qt in range(NQUAD):
            u0 = qt * UQ
            nu = min(UQ, N_UNITS - u0)
            cols = np.concatenate(
                [q8, tts[m][:, u0 * PAIR_W:(u0 + nu) * PAIR_W]], axis=1)
            nt = 2 + 2 * nu
            packed = cols.reshape(2, 32, nt, N_TILE).transpose(1, 2, 0, 3)
            full[32 * qt:32 * (qt + 1), :nt] = packed
        tqs.append(full)

    nc = _build_nc()
    in_maps = [{"tq": tqs[m]} for m in range(CORES)]
    res = run_bass_kernel_spmd(nc, in_maps, core_ids=list(range(CORES)))
    gmax = np.stack([np.asarray(r["gm"]).astype(np.float32)
                     for r in res.results])              # [8, B, GM_W]

    # ---- host stage 2: rank octets by distance lower bound ----
    gmax = np.nan_to_num(gmax, nan=-1e9, posinf=-1e9, neginf=-1e9)
    gmax_oct = gmax[:, :, oct2cols[0]]
    for j in range(1, 8):
        gmax_oct = np.maximum(gmax_oct, gmax[:, :, oct2cols[j]])  # [8,B,N_OCT]
    invalid_g = gnorm_min >= np.float32(1e9)         # [8, N_OCT]
    gmax_oct = np.where(invalid_g[:, None, :], np.float32(-1e9), gmax_oct)
    lb = gnorm_min[:, None, :] - 2.0 * gmax_oct      # [8, B, N_OCT]
    lb = lb.transpose(1, 0, 2).reshape(B, CORES * N_OCT)
    top_g = np.argpartition(lb, N_GROUPS, axis=1)[:, :N_GROUPS]

    core_of = top_g // N_OCT
    g_of = top_g % N_OCT
    rows = cand_rows[core_of, g_of].reshape(B, N_GROUPS * G)
    invalid = rows < 0
    rows_safe = np.where(invalid, 0, rows)

    # ---- exact rescore with the reference's formula (f32) ----
    tc_ = tk[rows_safe]                               # [B, NCAND, D]
    qn = np.einsum("ij,ij->i", q, q)
    tn = np.einsum("ij,ij->i", tk, tk)[rows_safe]
    dots = np.einsum("bd,bkd->bk", q, tc_)
    d2 = qn[:, None] - 2.0 * dots + tn
    d2 = np.where(invalid, np.float32(np.inf), d2).astype(np.float32)

    top_k = np.argpartition(d2, K, axis=1)[:, :K]
    rows_k = np.take_along_axis(rows_safe, top_k, axis=1)

    # ---- reference tail: exact sq, inverse-distance weights ----
    nb = tk[rows_k]
    sq = np.sum((q[:, None, :] - nb) ** 2, axis=2, dtype=np.float32)
    w = np.float32(1.0) / (sq + np.float32(DELTA))
    w = w / np.sum(w, axis=1, keepdims=True)
    out = np.sum(w * v[rows_k], axis=1)
    return out.astype(np.float32)


# revision 29
# speedup vs baseline: 1.3251x; 1.0013x over previous
"""Distributed brute-force kNN (top-50 inverse-distance-weighted regression), v4.

Strategy (8 NeuronCores):
  - Table (500k x 64) norm-sorted and striped across 8 cores (62500 rows each,
    padded to 63488 = 62 units x 1024 cols). fp8e4m3 everywhere on device;
    dots via DoubleRow matmuls (K packed as [32 partitions x 2 k-tiles]).
  - Octet-major device layout: within a unit, device col = rank (= octet*8 +
    member). Per 128-query group the 62 units' PSUM dots drain through the
    only two engines that can legally read PSUM (walrus allows ONE PSUM
    operand per instruction; Pool/DMA cannot touch PSUM at all, and Pool has
    no working tensor ops in this toolchain):
      D (29/62): DVE tensor_reduce straight from PSUM -- full octet max to
        128 fp8 gm cols per unit (1192ns). Unit 61 is half padding (ranks
        >= 62500), so it runs one matmul and a half-width reduce (658ns).
      A (33/62): ACT evicts PSUM straight to 1024 raw fp8 gm cols per unit
        (1038ns); the host maxes the 8 members per octet.
    Both engines run ~94% busy; this split balances them.
  - gm staging tiles hold a PAIR of qgs ([P, 2*(W+1)] with a pad col per
    half) so the 16 DMA procs (8 hwdge + 8 swdge) suffice: every DMA sits on
    a fresh proc and carries at most ONE wait (its writer RAW; walrus allows
    a single semaphore wait per instruction). hwdge: 4 input chunks + 3
    D-pair ships + the qg6 D half; swdge: 3 A-pair ships + the qg6 A half +
    qg7 tail chunks (3 for A, smallest last, + the whole D block).
  - Host: rank octets by the distance lower bound min_norm - 2*gmax, take
    top N_GROUPS=256 octets, re-score the 8*N_GROUPS candidates exactly in
    f32, final top-50 + inverse-distance weights (reference formula).

Sync discipline: per-engine total order via nosync deps (link()); every
cross-engine hazard is carried by exactly one instruction wait. Absorber ops
pull engine clocks forward so Tile never emits a second wait: standalone PE
ldweights absorb PSUM-slot WARs and input-chunk RAWs; at pair-tile recycling
boundaries a PUMP (reads a cell the engine wrote last qg; one self-sem wait
pulling the engine's observed self-clock current) precedes the tile
allocation and a CATCHER (writes the pad col the recycled tile's ship read,
sourced from a scratch cell written once at qg0) takes the ship-DMA WAR as
its only wait.
"""

import numpy as np
import ml_dtypes

import concourse.bass as bass
import concourse.tile as tile
from concourse import mybir
from concourse.bass_utils import run_bass_kernel_spmd
from contextlib import ExitStack

# Problem geometry (hardcoded per spec).
B = 1024          # queries
D = 64            # feature dim
C = 500000        # table capacity
K = 50            # neighbours
DELTA = 1e-3

CORES = 8
N_TILE = 512      # matmul free dim (one PSUM bank)
PAIR_W = 2 * N_TILE
N_UNITS = 62      # 62 * 1024 = 63488 >= 62500 real rows per core
N_COLS = N_UNITS * PAIR_W
C_LOC = 63488     # table width in tq (= N_COLS exactly)
G = 8             # octet size
OCT_W = PAIR_W // G            # 128 octets per unit
N_OCT = N_UNITS * OCT_W        # 7936 octets per core
P = 128           # partition dim == query-group size
QG = B // P       # 8 query groups

N_GROUPS = 256    # candidate octets per query taken on host

# unit kinds: D = DVE tensor_reduce straight from PSUM (full octet fold to
# 128 cols; walrus allows just ONE PSUM operand per instruction, so the
# two-input PSUM folds are illegal); A = ACT evicts PSUM straight to fp8 gm
# cols (1024/unit; the host maxes the 8 members -- Pool/gpsimd has no
# working tensor ops in this toolchain, so there is no cheap fold stage).
# Counts balance DVE (1192ns/unit) vs ACT (1038ns/unit).
N_D = 29
N_A = 33


def _mk_units(n_d=N_D, n_a=N_A):
    # largest-remainder interleave for smooth engine load
    counts = {k: v for k, v in (("D", n_d), ("A", n_a)) if v > 0}
    acc = {k: 0.0 for k in counts}
    out = []
    for i in range(N_UNITS):
        for k in counts:
            acc[k] += counts[k] / N_UNITS
        k = max(acc, key=lambda kk: acc[kk])
        acc[k] -= 1.0
        out.append(k)
    i0 = out.index("D")
    out = out[i0:] + out[:i0]
    assert all(out.count(k) == counts.get(k, 0) for k in ("D", "A"))
    return out


UNITS = _mk_units()
# unit 61 must be a D unit (its half-width saving lands on the binding DVE)
if UNITS[61] != "D":
    _j = max(i for i, k in enumerate(UNITS) if k == "D")
    UNITS[61], UNITS[_j] = UNITS[_j], UNITS[61]
DB_W = (N_D - 1) * OCT_W + OCT_W // 2 + 1   # D block + pad (u61 is half)
AB_W = N_A * PAIR_W + 1                 # A block + pad col (per qg half)
GM_W = DB_W + AB_W                      # DRAM gm row width
GM_DT = mybir.dt.float8e4
GM_NP_DT = ml_dtypes.float8_e4m3

# qg7 tail ship boundaries (A-unit index); the final chunk is smallest so
# the drain tail is short. The D block is small enough to ship whole.
A_TAIL_AT = (18, 29, N_A)

# quadrant layout: table third qt (21 units) lives on partitions
# [32qt, 32qt+32) with its own query replica (PE row-group bases 0/32/64;
# base 96 = broken HW quadrant).
UQ = 21
NQUAD = 3
NT_Q = 2 + 2 * UQ                  # 44 512-col tiles per third (2 = queries)

# input chunks (SP hwdge, procs 0-3): (part_lo, part_hi, tile_lo, tile_hi,
# first unit that needs it). Chunk 0 = queries + unit 0 only so compute
# starts as early as possible; thirds 1+2 merge into one 64-partition chunk
# to stay within 4 hwdge procs. The PE absorber for chunk k sits before
# unit need_u.
IN_CHUNKS = ((0, 32, 0, 4, 0), (0, 32, 4, 12, 1), (0, 32, 12, NT_Q, 5),
             (32, 96, 0, NT_Q, 21))

# unit 61 covers ranks [62464, 63488) but only 36 rows (< 62500) are real:
# its whole second 512-tile is padding, so it runs one matmul and a
# half-width tensor_reduce (658ns instead of 1192 on the binding DVE).
HALF_U = 61
HALF_OCT = OCT_W // 2

_NC_CACHE = {}


def _build_nc():
    if "nc" in _NC_CACHE:
        return _NC_CACHE["nc"]
    # swdge descriptor carveout sized so the ring never wraps (no ring waits)
    nc = bass.Bass(dynamic_dma_scratch_size=24576)
    # queries FIRST so the first weight loads only need the first DMA chunk
    # k-tiles interleaved per 512-col tile so every AP's k-step is 512
    tq = nc.declare_dram_parameter("tq", [96, NT_Q, 2, N_TILE],
                                   mybir.dt.float8e4, isOutput=False)
    gm = nc.declare_dram_parameter("gm", [B, GM_W], GM_DT, isOutput=True)

    from concourse.bass import _add_dep_helper as dep

    # Split TileContext's exit drain (one wait per active proc) into
    # one-wait-per-drain instructions.
    from concourse.vector_clock import VectorClock, ScopedClock

    def _split_drain_and_barrier(self, tick_clock, wait_clock):
        gc = tick_clock.global_clock
        for proc in range(27):
            t = gc.peek_next(proc) - 1
            if t <= 0:
                continue
            d = self.nc.sync.drain()
            pc = VectorClock()
            pc.require_at_least(proc, t)
            wait_clock.add_sem_waits(d.ins, ScopedClock({None: pc}))
        self.nc.all_engine_barrier()
        assert self.sems is not None
        popped = self.nc._tile_sem_poison_stack.pop()
        assert popped is self._sem_poison
        self.nc.clear_and_free_semaphores(list(self.sems.allocated().values()))
        self.nc.all_engine_barrier()

    tile.TileContext._drain_and_barrier = _split_drain_and_barrier

    with ExitStack() as ctx:
        tc = ctx.enter_context(tile.TileContext(nc, pool_alloc_mode="stack"))
        singles = ctx.enter_context(tc.tile_pool(name="singles", bufs=1))
        # one PSUM pool per consumer engine: within a tag all releases come
        # from one engine in issue order, so slot reuse is deterministic
        ppoolD = ctx.enter_context(tc.tile_pool(name="ppoolD", bufs=2, space="PSUM"))
        ppoolA = ctx.enter_context(tc.tile_pool(name="ppoolA", bufs=2, space="PSUM"))
        gmpoolD = ctx.enter_context(tc.tile_pool(name="gmpoolD", bufs=2))
        gmpoolA = ctx.enter_context(tc.tile_pool(name="gmpoolA", bufs=2))
        apool = ctx.enter_context(tc.tile_pool(name="apool", bufs=2))

        # chunked table load on the SP hwdge ring (procs 0-3): SP has no
        # engine work, hwdge descriptor gen is off the compute engines, and
        # the swdge procs stay fresh for the A/tail ships.
        t_sb = singles.tile([96, NT_Q, 2, N_TILE], mybir.dt.float8e4)
        for (pl, ph, tl, th, _nu) in IN_CHUNKS:
            nc.sync.dma_start(out=t_sb[pl:ph, tl:th], in_=tq[pl:ph, tl:th])

        chain = {"PE": None, "DVE": None, "ACT": None, "POOL": None}

        def link(key, inst):
            # total order per engine queue (add_dep_helper(a, b) == a waits b)
            if chain[key] is not None:
                dep(inst.ins, chain[key].ins, sync=False, reason=f"{key} order")
            chain[key] = inst
            return inst

        dscr = apool.tile([1, 1], GM_DT, tag="dscr")    # DVE pump sink
        ascr = apool.tile([1, 1], GM_DT, tag="ascr")    # ACT pump sink
        # catcher sources, written ONCE at qg0: later reads are ancient
        # same-engine RAWs already covered by the pump's self-clock
        dzero = apool.tile([1, 1], GM_DT, tag="dzero")
        azero4 = apool.tile([1, 4], GM_DT, tag="azero")
        azero = azero4[0:1, 0:1]
        link("DVE", nc.vector.memset(dzero, 0.0))
        link("ACT", nc.scalar.memzero(azero4))

        d_srcs = []            # gm cell AP per D-unit (PE absorber source)
        a_srcs = []            # gm cell AP per A-unit (PE absorber source)
        nd = 0                 # global D-unit counter
        na = 0                 # global A-unit counter
        gmD = gmA = None

        ck_i = 0
        for qg in range(QG):
            qq = (qg % 4) * P
            half = qg % 2
            dh = half * DB_W            # col base of this qg's half in gmD
            ah = half * AB_W            # col base of this qg's half in gmA
            if half == 0:
                if qg >= 4:
                    # pair-tile recycling: PUMP (reads the previous pair
                    # tile's odd-half cell -- recent self-RAW pulling the
                    # engine's self-clock to the present), then allocate, then
                    # CATCHER (takes the recycled tile's pair-ship WAR via the
                    # pad col -- its single wait).
                    link("DVE", nc.vector.tensor_copy(
                        dscr, gmD[0:1, DB_W:DB_W + 1]))
                    link("ACT", nc.scalar.copy(
                        ascr, gmA[0:1, AB_W:AB_W + 1]))
                    gmD = gmpoolD.tile([P, 2 * DB_W], GM_DT, tag="gmD")
                    link("DVE", nc.vector.tensor_copy(
                        gmD[0:1, DB_W - 1:DB_W], dzero))
                    gmA = gmpoolA.tile([P, 2 * AB_W], GM_DT, tag="gmA")
                    link("ACT", nc.scalar.copy(
                        gmA[0:1, AB_W - 1:AB_W], azero))
                else:
                    gmD = gmpoolD.tile([P, 2 * DB_W], GM_DT, tag="gmD")
                    gmA = gmpoolA.tile([P, 2 * AB_W], GM_DT, tag="gmA")

            di = 0             # D-units completed within this qg
            ai = 0             # A-units completed within this qg
            a_tail_i = 0
            for u, kind in enumerate(UNITS):
                qt = u // UQ            # third (partition base 32*qt)
                ps = slice(32 * qt, 32 * (qt + 1))
                ut = 2 + 2 * (u % UQ)   # first 512-tile of unit u in third
                lhsT = t_sb[ps, qg // 4, :, qq:qq + P]
                if qg == 0 and ck_i < len(IN_CHUNKS) and u == IN_CHUNKS[ck_i][4]:
                    # absorb this chunk's DMA-complete wait into a ldweights
                    cpl, cph, ctl, cth, _nu = IN_CHUNKS[ck_i]
                    link("PE", nc.tensor.ldweights(
                        weights=t_sb[cph - 32:cph, cth - 1, :, N_TILE - 1:N_TILE],
                        tile_position=(cph - 32, 0)))
                    ck_i += 1

                # PE absorber: standalone ldweights (no output; clobbered by
                # the next matmul's weight load) reading the output of the
                # consumer that released this unit's PSUM slot.
                if kind == "D":
                    dsrc = d_srcs[nd - 2] if nd >= 2 else t_sb[0:32, 0, 0, 0:1]
                else:
                    dsrc = a_srcs[na - 2] if na >= 2 else t_sb[0:32, 0, 0, 0:1]
                link("PE", nc.tensor.ldweights(weights=dsrc))

                pool_, tag_ = (ppoolD, "ppD") if kind == "D" else (ppoolA, "ppA")
                p = pool_.tile([P, PAIR_W], mybir.dt.float32, tag=tag_)
                nmm = 1 if u == HALF_U else 2
                for j in range(nmm):
                    link("PE", nc.tensor.matmul(
                        p[:, j * N_TILE:(j + 1) * N_TILE], lhsT,
                        t_sb[ps, ut + j], start=True, stop=True,
                        perf_mode=mybir.MatmulPerfMode.DoubleRow))

                pv = p.rearrange("p (o m) -> p o m", m=G)
                if kind == "D":
                    w = HALF_OCT if u == HALF_U else OCT_W
                    gbase = dh + di * OCT_W
                    link("DVE", nc.vector.tensor_reduce(
                        out=gmD[:, gbase:gbase + w], in_=pv[:, 0:w, :],
                        axis=mybir.AxisListType.X, op=mybir.AluOpType.max))
                    d_srcs.append(gmD[0:D, gbase:gbase + 1])
                    nd += 1
                    di += 1
                    if qg == QG - 1 and di == N_D:
                        # qg7 D block ships whole on a swdge proc
                        link("POOL", nc.gpsimd.dma_start(
                            out=gm[qg * P:(qg + 1) * P, 0:DB_W],
                            in_=gmD[:, dh:dh + DB_W]))
                else:
                    gbase = ah + ai * PAIR_W
                    link("ACT", nc.scalar.copy(
                        gmA[:, gbase:gbase + PAIR_W], p))
                    a_srcs.append(gmA[0:D, gbase:gbase + 1])
                    na += 1
                    ai += 1
                    if qg == QG - 1 and ai == A_TAIL_AT[a_tail_i]:
                        lo = (A_TAIL_AT[a_tail_i - 1] if a_tail_i else 0) * PAIR_W
                        hi = ai * PAIR_W if a_tail_i < len(A_TAIL_AT) - 1 else AB_W
                        link("POOL", nc.gpsimd.dma_start(
                            out=gm[qg * P:(qg + 1) * P, DB_W + lo:DB_W + hi],
                            in_=gmA[:, ah + lo:ah + hi]))
                        a_tail_i += 1

            if qg % 2 == 1 and qg < QG - 1:
                # pair ships (two qgs' rows in one DMA): D via SP hwdge, A via
                # the Pool swdge queue; one RAW wait each (single writer)
                rows = gm[(qg - 1) * P:(qg + 1) * P]
                nc.sync.dma_start(
                    out=rows[:, 0:DB_W].rearrange("(h p) c -> p h c", h=2),
                    in_=gmD.rearrange("p (h c) -> p h c", h=2))
                link("POOL", nc.gpsimd.dma_start(
                    out=rows[:, DB_W:GM_W].rearrange("(h p) c -> p h c", h=2),
                    in_=gmA.rearrange("p (h c) -> p h c", h=2)))
            elif qg == QG - 2:
                # qg6: ship this half now (qg7 ships at its end); D on the
                # last hwdge proc, A on a swdge proc
                nc.sync.dma_start(
                    out=gm[qg * P:(qg + 1) * P, 0:DB_W], in_=gmD[:, 0:DB_W])
                link("POOL", nc.gpsimd.dma_start(
                    out=gm[qg * P:(qg + 1) * P, DB_W:GM_W], in_=gmA[:, 0:AB_W]))

    # wait audit: walrus accepts at most one semaphore wait per instruction
    bad = []
    for blk in nc.m.functions[0].blocks:
        for inst in blk.instructions:
            si = inst.sync_info
            if si is None:
                continue
            nw = len(si.on_wait or [])
            if nw > 1:
                bad.append((type(inst).__name__, inst.name, nw))
    assert not bad, f"multi-wait instructions: {bad[:10]}"

    _NC_CACHE["nc"] = nc
    return nc


def _preprocess(table_keys):
    """Norm-sort, stripe across cores; octet-major device layout (device col
    within a unit = local rank, octet o covers ranks [8*(128u+o), ...+8)).
    gm holds 4 cols per octet (member-pair maxima from fold1); the host maxes
    them. Returns per-core fp8 tables plus octet-indexed candidate rows /
    min-norms and the octet -> gm column map."""
    tk = np.ascontiguousarray(table_keys, dtype=np.float32)
    norms = np.einsum("ij,ij->i", tk, tk)
    order = np.argsort(norms, kind="stable")

    # gm col base per unit: D block cols [0, N_D*OCT_W), pad, A block
    colbase = np.zeros(N_UNITS, dtype=np.int64)
    kinds = []
    dnext, anext = 0, DB_W
    for u, kind in enumerate(UNITS):
        kinds.append(kind)
        if kind == "D":
            colbase[u] = dnext
            dnext += OCT_W
        else:
            colbase[u] = anext
            anext += PAIR_W

    # octet id (u*128+o) -> its gm columns: D units have one col per octet
    # (repeated 8x), A units have the 8 raw member cols (host maxes them)
    oct2cols = np.zeros((8, N_OCT), dtype=np.int64)
    o = np.arange(OCT_W)
    for u in range(N_UNITS):
        oid = u * OCT_W + o
        for j in range(8):
            if kinds[u] == "D":
                # u61's octets beyond its half width are padding (rank >=
                # 62500, masked invalid by gnorm_min); clamp their col
                oc = np.minimum(o, OCT_W // 2 - 1) if u == 61 else o
                oct2cols[j, oid] = colbase[u] + oc
            else:
                oct2cols[j, oid] = colbase[u] + 8 * o + j

    tts = []
    cand_rows = np.full((CORES, N_OCT, G), -1, dtype=np.int64)
    gnorm_min = np.full((CORES, N_OCT), np.float32(1e9), dtype=np.float32)

    n_loc = C // CORES             # 62500 real rows per core

    for m in range(CORES):
        rows_m = order[m::CORES]
        Tdev = np.zeros((N_COLS, D), dtype=np.float32)   # rank == device col
        Tdev[:n_loc] = tk[rows_m]
        nrm = np.full(N_COLS, np.float32(1e9), dtype=np.float32)
        nrm[:n_loc] = norms[rows_m]

        oid = np.arange(N_OCT)
        ranks = oid[:, None] * G + np.arange(G)[None, :]
        valid = ranks < n_loc
        ranks_c = np.minimum(ranks, n_loc - 1)
        cand_rows[m] = np.where(valid, rows_m[ranks_c], -1)
        gnorm_min[m] = np.where(valid[:, 0], nrm[ranks[:, 0]], np.float32(1e9))

        tts.append(Tdev.T.astype(ml_dtypes.float8_e4m3))   # [64, C_LOC]

    return tts, cand_rows, gnorm_min, oct2cols


def kernel(keys, table_keys, table_values):
    q = np.ascontiguousarray(keys, dtype=np.float32)
    tk = np.ascontiguousarray(table_keys, dtype=np.float32)
    v = np.ascontiguousarray(table_values, dtype=np.float32)

    tts, cand_rows, gnorm_min, oct2cols = _preprocess(tk)
    q8 = q.T.astype(ml_dtypes.float8_e4m3)                 # [64, B]
    # per third: [64, (2 + 42)*512 cols] -> [32, NT_Q, 2, 512]; thirds
    # stacked on the partition axis -> [96, NT_Q, 2, 512]
    tqs = []
    for m in range(CORES):
        full = np.zeros((96, NT_Q, 2, N_TILE), dtype=ml_dtypes.float8_e4m3)
        for qt in range(NQUAD):
            u0 = qt * UQ
            nu = min(UQ, N_UNITS - u0)
            cols = np.concatenate(
                [q8, tts[m][:, u0 * PAIR_W:(u0 + nu) * PAIR_W]], axis=1)
            nt = 2 + 2 * nu
            packed = cols.reshape(2, 32, nt, N_TILE).transpose(1, 2, 0, 3)
            full[32 * qt:32 * (qt + 1), :nt] = packed
        tqs.append(full)

    nc = _build_nc()
    in_maps = [{"tq": tqs[m]} for m in range(CORES)]
    res = run_bass_kernel_spmd(nc, in_maps, core_ids=list(range(CORES)))
    gmax = np.stack([np.asarray(r["gm"]).astype(np.float32)
                     for r in res.results])              # [8, B, GM_W]

    # ---- host stage 2: rank octets by distance lower bound ----
    gmax = np.nan_to_num(gmax, nan=-1e9, posinf=-1e9, neginf=-1e9)
    gmax_oct = gmax[:, :, oct2cols[0]]
    for j in range(1, 8):
        gmax_oct = np.maximum(gmax_oct, gmax[:, :, oct2cols[j]])  # [8,B,N_OCT]
    invalid_g = gnorm_min >= np.float32(1e9)         # [8, N_OCT]
    gmax_oct = np.where(invalid_g[:, None, :], np.float32(-1e9), gmax_oct)
    lb = gnorm_min[:, None, :] - 2.0 * gmax_oct      # [8, B, N_OCT]
    lb = lb.transpose(1, 0, 2).reshape(B, CORES * N_OCT)
    top_g = np.argpartition(lb, N_GROUPS, axis=1)[:, :N_GROUPS]

    core_of = top_g // N_OCT
    g_of = top_g % N_OCT
    rows = cand_rows[core_of, g_of].reshape(B, N_GROUPS * G)
    invalid = rows < 0
    rows_safe = np.where(invalid, 0, rows)

    # ---- exact rescore with the reference's formula (f32) ----
    tc_ = tk[rows_safe]                               # [B, NCAND, D]
    qn = np.einsum("ij,ij->i", q, q)
    tn = np.einsum("ij,ij->i", tk, tk)[rows_safe]
    dots = np.einsum("bd,bkd->bk", q, tc_)
    d2 = qn[:, None] - 2.0 * dots + tn
    d2 = np.where(invalid, np.float32(np.inf), d2).astype(np.float32)

    top_k = np.argpartition(d2, K, axis=1)[:, :K]
    rows_k = np.take_along_axis(rows_safe, top_k, axis=1)

    # ---- reference tail: exact sq, inverse-distance weights ----
    nb = tk[rows_k]
    sq = np.sum((q[:, None, :] - nb) ** 2, axis=2, dtype=np.float32)
    w = np.float32(1.0) / (sq + np.float32(DELTA))
    w = w / np.sum(w, axis=1, keepdims=True)
    out = np.sum(w * v[rows_k], axis=1)
    return out.astype(np.float32)


# revision 30
# speedup vs baseline: 1.3292x; 1.0031x over previous
"""Distributed brute-force kNN (top-50 inverse-distance-weighted regression), v4.

Strategy (8 NeuronCores):
  - Table (500k x 64) norm-sorted and striped across 8 cores (62500 rows each,
    padded to 63488 = 62 units x 1024 cols). fp8e4m3 everywhere on device;
    dots via DoubleRow matmuls (K packed as [32 partitions x 2 k-tiles]).
  - Octet-major device layout: within a unit, device col = rank (= octet*8 +
    member). Per 128-query group the 62 units' PSUM dots drain through the
    only two engines that can legally read PSUM (walrus allows ONE PSUM
    operand per instruction; Pool/DMA cannot touch PSUM at all, and Pool has
    no working tensor ops in this toolchain):
      D (29/62): DVE tensor_reduce straight from PSUM -- full octet max to
        128 fp8 gm cols per unit (1192ns). Unit 61 is half padding (ranks
        >= 62500), so it runs one matmul and a half-width reduce (658ns).
      A (33/62): ACT evicts PSUM straight to 1024 raw fp8 gm cols per unit
        (1038ns); the host maxes the 8 members per octet.
    Both engines run ~94% busy; this split balances them.
  - gm staging tiles hold a PAIR of qgs ([P, 2*(W+1)] with a pad col per
    half) so the 16 DMA procs (8 hwdge + 8 swdge) suffice: every DMA sits on
    a fresh proc and carries at most ONE wait (its writer RAW; walrus allows
    a single semaphore wait per instruction). hwdge: 4 input chunks + 3
    D-pair ships + the qg6 D half; swdge: 3 A-pair ships + the qg6 A half +
    qg7 tail chunks (3 for A, smallest last, + the whole D block).
  - Host: rank octets by the distance lower bound min_norm - 2*gmax, take
    top N_GROUPS=256 octets, re-score the 8*N_GROUPS candidates exactly in
    f32, final top-50 + inverse-distance weights (reference formula).

Sync discipline: per-engine total order via nosync deps (link()); every
cross-engine hazard is carried by exactly one instruction wait. Absorber ops
pull engine clocks forward so Tile never emits a second wait: standalone PE
ldweights absorb PSUM-slot WARs and input-chunk RAWs; at pair-tile recycling
boundaries a PUMP (reads a cell the engine wrote last qg; one self-sem wait
pulling the engine's observed self-clock current) precedes the tile
allocation and a CATCHER (writes the pad col the recycled tile's ship read,
sourced from a scratch cell written once at qg0) takes the ship-DMA WAR as
its only wait.
"""

import numpy as np
import ml_dtypes

import concourse.bass as bass
import concourse.tile as tile
from concourse import mybir
from concourse.bass_utils import run_bass_kernel_spmd
from contextlib import ExitStack

# Problem geometry (hardcoded per spec).
B = 1024          # queries
D = 64            # feature dim
C = 500000        # table capacity
K = 50            # neighbours
DELTA = 1e-3

CORES = 8
N_TILE = 512      # matmul free dim (one PSUM bank)
PAIR_W = 2 * N_TILE
N_UNITS = 62      # 62 * 1024 = 63488 >= 62500 real rows per core
N_COLS = N_UNITS * PAIR_W
C_LOC = 63488     # table width in tq (= N_COLS exactly)
G = 8             # octet size
OCT_W = PAIR_W // G            # 128 octets per unit
N_OCT = N_UNITS * OCT_W        # 7936 octets per core
P = 128           # partition dim == query-group size
QG = B // P       # 8 query groups

N_GROUPS = 256    # candidate octets per query taken on host

# unit kinds: D = DVE tensor_reduce straight from PSUM (full octet fold to
# 128 cols; walrus allows just ONE PSUM operand per instruction, so the
# two-input PSUM folds are illegal); A = ACT evicts PSUM straight to fp8 gm
# cols (1024/unit; the host maxes the 8 members -- Pool/gpsimd has no
# working tensor ops in this toolchain, so there is no cheap fold stage).
# Counts balance DVE (1192ns/unit) vs ACT (1038ns/unit).
N_D = 29
N_A = 33


def _mk_units(n_d=N_D, n_a=N_A):
    # largest-remainder interleave for smooth engine load
    counts = {k: v for k, v in (("D", n_d), ("A", n_a)) if v > 0}
    acc = {k: 0.0 for k in counts}
    out = []
    for i in range(N_UNITS):
        for k in counts:
            acc[k] += counts[k] / N_UNITS
        k = max(acc, key=lambda kk: acc[kk])
        acc[k] -= 1.0
        out.append(k)
    i0 = out.index("D")
    out = out[i0:] + out[:i0]
    assert all(out.count(k) == counts.get(k, 0) for k in ("D", "A"))
    return out


UNITS = _mk_units()
# unit 61 must be a D unit (its half-width saving lands on the binding DVE)
if UNITS[61] != "D":
    _j = max(i for i, k in enumerate(UNITS) if k == "D")
    UNITS[61], UNITS[_j] = UNITS[_j], UNITS[61]
DB_W = (N_D - 1) * OCT_W + OCT_W // 2 + 1   # D block + pad (u61 is half)
AB_W = N_A * PAIR_W + 1                 # A block + pad col (per qg half)
GM_W = DB_W + AB_W                      # DRAM gm row width
GM_DT = mybir.dt.float8e4
GM_NP_DT = ml_dtypes.float8_e4m3

# qg7 tail ship boundaries (A-unit index); the final chunk is smallest so
# the drain tail is short. The D block is small enough to ship whole.
A_TAIL_AT = (18, 29, N_A)

# quadrant layout: table third qt (21 units) lives on partitions
# [32qt, 32qt+32) with its own query replica (PE row-group bases 0/32/64;
# base 96 = broken HW quadrant).
UQ = 21
NQUAD = 3
NT_Q = 2 + 2 * UQ                  # 44 512-col tiles per third (2 = queries)

# input chunks (SP hwdge, procs 0-3): (part_lo, part_hi, tile_lo, tile_hi,
# first unit that needs it). Chunk 0 = queries + unit 0 only so compute
# starts as early as possible; thirds 1+2 merge into one 64-partition chunk
# to stay within 4 hwdge procs. The PE absorber for chunk k sits before
# unit need_u.
IN_CHUNKS = ((0, 32, 0, 6, 0), (0, 32, 6, 16, 2), (0, 32, 16, NT_Q, 7),
             (32, 96, 0, NT_Q, 21))

# unit 61 covers ranks [62464, 63488) but only 36 rows (< 62500) are real:
# its whole second 512-tile is padding, so it runs one matmul and a
# half-width tensor_reduce (658ns instead of 1192 on the binding DVE).
HALF_U = 61
HALF_OCT = OCT_W // 2

_NC_CACHE = {}


def _build_nc():
    if "nc" in _NC_CACHE:
        return _NC_CACHE["nc"]
    # swdge descriptor carveout sized so the ring never wraps (no ring waits)
    nc = bass.Bass(dynamic_dma_scratch_size=24576)
    # queries FIRST so the first weight loads only need the first DMA chunk
    # k-tiles interleaved per 512-col tile so every AP's k-step is 512
    tq = nc.declare_dram_parameter("tq", [96, NT_Q, 2, N_TILE],
                                   mybir.dt.float8e4, isOutput=False)
    gm = nc.declare_dram_parameter("gm", [B, GM_W], GM_DT, isOutput=True)

    from concourse.bass import _add_dep_helper as dep

    # Split TileContext's exit drain (one wait per active proc) into
    # one-wait-per-drain instructions.
    from concourse.vector_clock import VectorClock, ScopedClock

    def _split_drain_and_barrier(self, tick_clock, wait_clock):
        gc = tick_clock.global_clock
        for proc in range(27):
            t = gc.peek_next(proc) - 1
            if t <= 0:
                continue
            d = self.nc.sync.drain()
            pc = VectorClock()
            pc.require_at_least(proc, t)
            wait_clock.add_sem_waits(d.ins, ScopedClock({None: pc}))
        self.nc.all_engine_barrier()
        assert self.sems is not None
        popped = self.nc._tile_sem_poison_stack.pop()
        assert popped is self._sem_poison
        self.nc.clear_and_free_semaphores(list(self.sems.allocated().values()))
        self.nc.all_engine_barrier()

    tile.TileContext._drain_and_barrier = _split_drain_and_barrier

    with ExitStack() as ctx:
        tc = ctx.enter_context(tile.TileContext(nc, pool_alloc_mode="stack"))
        singles = ctx.enter_context(tc.tile_pool(name="singles", bufs=1))
        # one PSUM pool per consumer engine: within a tag all releases come
        # from one engine in issue order, so slot reuse is deterministic
        ppoolD = ctx.enter_context(tc.tile_pool(name="ppoolD", bufs=2, space="PSUM"))
        ppoolA = ctx.enter_context(tc.tile_pool(name="ppoolA", bufs=2, space="PSUM"))
        gmpoolD = ctx.enter_context(tc.tile_pool(name="gmpoolD", bufs=2))
        gmpoolA = ctx.enter_context(tc.tile_pool(name="gmpoolA", bufs=2))
        apool = ctx.enter_context(tc.tile_pool(name="apool", bufs=2))

        # chunked table load on the SP hwdge ring (procs 0-3): SP has no
        # engine work, hwdge descriptor gen is off the compute engines, and
        # the swdge procs stay fresh for the A/tail ships.
        t_sb = singles.tile([96, NT_Q, 2, N_TILE], mybir.dt.float8e4)
        for (pl, ph, tl, th, _nu) in IN_CHUNKS:
            nc.sync.dma_start(out=t_sb[pl:ph, tl:th], in_=tq[pl:ph, tl:th])

        chain = {"PE": None, "DVE": None, "ACT": None, "POOL": None}

        def link(key, inst):
            # total order per engine queue (add_dep_helper(a, b) == a waits b)
            if chain[key] is not None:
                dep(inst.ins, chain[key].ins, sync=False, reason=f"{key} order")
            chain[key] = inst
            return inst

        dscr = apool.tile([1, 1], GM_DT, tag="dscr")    # DVE pump sink
        ascr = apool.tile([1, 1], GM_DT, tag="ascr")    # ACT pump sink
        # catcher sources, written ONCE at qg0: later reads are ancient
        # same-engine RAWs already covered by the pump's self-clock
        dzero = apool.tile([1, 1], GM_DT, tag="dzero")
        azero4 = apool.tile([1, 4], GM_DT, tag="azero")
        azero = azero4[0:1, 0:1]
        link("DVE", nc.vector.memset(dzero, 0.0))
        link("ACT", nc.scalar.memzero(azero4))

        d_srcs = []            # gm cell AP per D-unit (PE absorber source)
        a_srcs = []            # gm cell AP per A-unit (PE absorber source)
        nd = 0                 # global D-unit counter
        na = 0                 # global A-unit counter
        gmD = gmA = None

        ck_i = 0
        for qg in range(QG):
            qq = (qg % 4) * P
            half = qg % 2
            dh = half * DB_W            # col base of this qg's half in gmD
            ah = half * AB_W            # col base of this qg's half in gmA
            if half == 0:
                if qg >= 4:
                    # pair-tile recycling: PUMP (reads the previous pair
                    # tile's odd-half cell -- recent self-RAW pulling the
                    # engine's self-clock to the present), then allocate, then
                    # CATCHER (takes the recycled tile's pair-ship WAR via the
                    # pad col -- its single wait).
                    link("DVE", nc.vector.tensor_copy(
                        dscr, gmD[0:1, DB_W:DB_W + 1]))
                    link("ACT", nc.scalar.copy(
                        ascr, gmA[0:1, AB_W:AB_W + 1]))
                    gmD = gmpoolD.tile([P, 2 * DB_W], GM_DT, tag="gmD")
                    link("DVE", nc.vector.tensor_copy(
                        gmD[0:1, DB_W - 1:DB_W], dzero))
                    gmA = gmpoolA.tile([P, 2 * AB_W], GM_DT, tag="gmA")
                    link("ACT", nc.scalar.copy(
                        gmA[0:1, AB_W - 1:AB_W], azero))
                else:
                    gmD = gmpoolD.tile([P, 2 * DB_W], GM_DT, tag="gmD")
                    gmA = gmpoolA.tile([P, 2 * AB_W], GM_DT, tag="gmA")

            di = 0             # D-units completed within this qg
            ai = 0             # A-units completed within this qg
            a_tail_i = 0
            for u, kind in enumerate(UNITS):
                qt = u // UQ            # third (partition base 32*qt)
                ps = slice(32 * qt, 32 * (qt + 1))
                ut = 2 + 2 * (u % UQ)   # first 512-tile of unit u in third
                lhsT = t_sb[ps, qg // 4, :, qq:qq + P]
                if qg == 0 and ck_i < len(IN_CHUNKS) and u == IN_CHUNKS[ck_i][4]:
                    # absorb this chunk's DMA-complete wait into a ldweights
                    cpl, cph, ctl, cth, _nu = IN_CHUNKS[ck_i]
                    link("PE", nc.tensor.ldweights(
                        weights=t_sb[cph - 32:cph, cth - 1, :, N_TILE - 1:N_TILE],
                        tile_position=(cph - 32, 0)))
                    ck_i += 1

                # PE absorber: standalone ldweights (no output; clobbered by
                # the next matmul's weight load) reading the output of the
                # consumer that released this unit's PSUM slot.
                if kind == "D":
                    dsrc = d_srcs[nd - 2] if nd >= 2 else t_sb[0:32, 0, 0, 0:1]
                else:
                    dsrc = a_srcs[na - 2] if na >= 2 else t_sb[0:32, 0, 0, 0:1]
                link("PE", nc.tensor.ldweights(weights=dsrc))

                pool_, tag_ = (ppoolD, "ppD") if kind == "D" else (ppoolA, "ppA")
                p = pool_.tile([P, PAIR_W], mybir.dt.float32, tag=tag_)
                nmm = 1 if u == HALF_U else 2
                for j in range(nmm):
                    link("PE", nc.tensor.matmul(
                        p[:, j * N_TILE:(j + 1) * N_TILE], lhsT,
                        t_sb[ps, ut + j], start=True, stop=True,
                        perf_mode=mybir.MatmulPerfMode.DoubleRow))

                pv = p.rearrange("p (o m) -> p o m", m=G)
                if kind == "D":
                    w = HALF_OCT if u == HALF_U else OCT_W
                    gbase = dh + di * OCT_W
                    link("DVE", nc.vector.tensor_reduce(
                        out=gmD[:, gbase:gbase + w], in_=pv[:, 0:w, :],
                        axis=mybir.AxisListType.X, op=mybir.AluOpType.max))
                    d_srcs.append(gmD[0:D, gbase:gbase + 1])
                    nd += 1
                    di += 1
                    if qg == QG - 1 and di == N_D:
                        # qg7 D block ships whole on a swdge proc
                        link("POOL", nc.gpsimd.dma_start(
                            out=gm[qg * P:(qg + 1) * P, 0:DB_W],
                            in_=gmD[:, dh:dh + DB_W]))
                else:
                    gbase = ah + ai * PAIR_W
                    link("ACT", nc.scalar.copy(
                        gmA[:, gbase:gbase + PAIR_W], p))
                    a_srcs.append(gmA[0:D, gbase:gbase + 1])
                    na += 1
                    ai += 1
                    if qg == QG - 1 and ai == A_TAIL_AT[a_tail_i]:
                        lo = (A_TAIL_AT[a_tail_i - 1] if a_tail_i else 0) * PAIR_W
                        hi = ai * PAIR_W if a_tail_i < len(A_TAIL_AT) - 1 else AB_W
                        link("POOL", nc.gpsimd.dma_start(
                            out=gm[qg * P:(qg + 1) * P, DB_W + lo:DB_W + hi],
                            in_=gmA[:, ah + lo:ah + hi]))
                        a_tail_i += 1

            if qg % 2 == 1 and qg < QG - 1:
                # pair ships (two qgs' rows in one DMA): D via SP hwdge, A via
                # the Pool swdge queue; one RAW wait each (single writer)
                rows = gm[(qg - 1) * P:(qg + 1) * P]
                nc.sync.dma_start(
                    out=rows[:, 0:DB_W].rearrange("(h p) c -> p h c", h=2),
                    in_=gmD.rearrange("p (h c) -> p h c", h=2))
                link("POOL", nc.gpsimd.dma_start(
                    out=rows[:, DB_W:GM_W].rearrange("(h p) c -> p h c", h=2),
                    in_=gmA.rearrange("p (h c) -> p h c", h=2)))
            elif qg == QG - 2:
                # qg6: ship this half now (qg7 ships at its end); D on the
                # last hwdge proc, A on a swdge proc
                nc.sync.dma_start(
                    out=gm[qg * P:(qg + 1) * P, 0:DB_W], in_=gmD[:, 0:DB_W])
                link("POOL", nc.gpsimd.dma_start(
                    out=gm[qg * P:(qg + 1) * P, DB_W:GM_W], in_=gmA[:, 0:AB_W]))

    # wait audit: walrus accepts at most one semaphore wait per instruction
    bad = []
    for blk in nc.m.functions[0].blocks:
        for inst in blk.instructions:
            si = inst.sync_info
            if si is None:
                continue
            nw = len(si.on_wait or [])
            if nw > 1:
                bad.append((type(inst).__name__, inst.name, nw))
    assert not bad, f"multi-wait instructions: {bad[:10]}"

    _NC_CACHE["nc"] = nc
    return nc


def _preprocess(table_keys):
    """Norm-sort, stripe across cores; octet-major device layout (device col
    within a unit = local rank, octet o covers ranks [8*(128u+o), ...+8)).
    gm holds 4 cols per octet (member-pair maxima from fold1); the host maxes
    them. Returns per-core fp8 tables plus octet-indexed candidate rows /
    min-norms and the octet -> gm column map."""
    tk = np.ascontiguousarray(table_keys, dtype=np.float32)
    norms = np.einsum("ij,ij->i", tk, tk)
    order = np.argsort(norms, kind="stable")

    # gm col base per unit: D block cols [0, N_D*OCT_W), pad, A block
    colbase = np.zeros(N_UNITS, dtype=np.int64)
    kinds = []
    dnext, anext = 0, DB_W
    for u, kind in enumerate(UNITS):
        kinds.append(kind)
        if kind == "D":
            colbase[u] = dnext
            dnext += OCT_W
        else:
            colbase[u] = anext
            anext += PAIR_W

    # octet id (u*128+o) -> its gm columns: D units have one col per octet
    # (repeated 8x), A units have the 8 raw member cols (host maxes them)
    oct2cols = np.zeros((8, N_OCT), dtype=np.int64)
    o = np.arange(OCT_W)
    for u in range(N_UNITS):
        oid = u * OCT_W + o
        for j in range(8):
            if kinds[u] == "D":
                # u61's octets beyond its half width are padding (rank >=
                # 62500, masked invalid by gnorm_min); clamp their col
                oc = np.minimum(o, OCT_W // 2 - 1) if u == 61 else o
                oct2cols[j, oid] = colbase[u] + oc
            else:
                oct2cols[j, oid] = colbase[u] + 8 * o + j

    tts = []
    cand_rows = np.full((CORES, N_OCT, G), -1, dtype=np.int64)
    gnorm_min = np.full((CORES, N_OCT), np.float32(1e9), dtype=np.float32)

    n_loc = C // CORES             # 62500 real rows per core

    for m in range(CORES):
        rows_m = order[m::CORES]
        Tdev = np.zeros((N_COLS, D), dtype=np.float32)   # rank == device col
        Tdev[:n_loc] = tk[rows_m]
        nrm = np.full(N_COLS, np.float32(1e9), dtype=np.float32)
        nrm[:n_loc] = norms[rows_m]

        oid = np.arange(N_OCT)
        ranks = oid[:, None] * G + np.arange(G)[None, :]
        valid = ranks < n_loc
        ranks_c = np.minimum(ranks, n_loc - 1)
        cand_rows[m] = np.where(valid, rows_m[ranks_c], -1)
        gnorm_min[m] = np.where(valid[:, 0], nrm[ranks[:, 0]], np.float32(1e9))

        tts.append(Tdev.T.astype(ml_dtypes.float8_e4m3))   # [64, C_LOC]

    return tts, cand_rows, gnorm_min, oct2cols


def kernel(keys, table_keys, table_values):
    q = np.ascontiguousarray(keys, dtype=np.float32)
    tk = np.ascontiguousarray(table_keys, dtype=np.float32)
    v = np.ascontiguousarray(table_values, dtype=np.float32)

    tts, cand_rows, gnorm_min, oct2cols = _preprocess(tk)
    q8 = q.T.astype(ml_dtypes.float8_e4m3)                 # [64, B]
    # per third: [64, (2 + 42)*512 cols] -> [32, NT_Q, 2, 512]; thirds
    # stacked on the partition axis -> [96, NT_Q, 2, 512]
    tqs = []
    for m in range(CORES):
        full = np.zeros((96, NT_Q, 2, N_TILE), dtype=ml_dtypes.float8_e4m3)
        for qt in range(NQUAD):
            u0 = qt * UQ
            nu = min(UQ, N_UNITS - u0)
            cols = np.concatenate(
                [q8, tts[m][:, u0 * PAIR_W:(u0 + nu) * PAIR_W]], axis=1)
            nt = 2 + 2 * nu
            packed = cols.reshape(2, 32, nt, N_TILE).transpose(1, 2, 0, 3)
            full[32 * qt:32 * (qt + 1), :nt] = packed
        tqs.append(full)

    nc = _build_nc()
    in_maps = [{"tq": tqs[m]} for m in range(CORES)]
    res = run_bass_kernel_spmd(nc, in_maps, core_ids=list(range(CORES)))
    gmax = np.stack([np.asarray(r["gm"]).astype(np.float32)
                     for r in res.results])              # [8, B, GM_W]

    # ---- host stage 2: rank octets by distance lower bound ----
    gmax = np.nan_to_num(gmax, nan=-1e9, posinf=-1e9, neginf=-1e9)
    gmax_oct = gmax[:, :, oct2cols[0]]
    for j in range(1, 8):
        gmax_oct = np.maximum(gmax_oct, gmax[:, :, oct2cols[j]])  # [8,B,N_OCT]
    invalid_g = gnorm_min >= np.float32(1e9)         # [8, N_OCT]
    gmax_oct = np.where(invalid_g[:, None, :], np.float32(-1e9), gmax_oct)
    lb = gnorm_min[:, None, :] - 2.0 * gmax_oct      # [8, B, N_OCT]
    lb = lb.transpose(1, 0, 2).reshape(B, CORES * N_OCT)
    top_g = np.argpartition(lb, N_GROUPS, axis=1)[:, :N_GROUPS]

    core_of = top_g // N_OCT
    g_of = top_g % N_OCT
    rows = cand_rows[core_of, g_of].reshape(B, N_GROUPS * G)
    invalid = rows < 0
    rows_safe = np.where(invalid, 0, rows)

    # ---- exact rescore with the reference's formula (f32) ----
    tc_ = tk[rows_safe]                               # [B, NCAND, D]
    qn = np.einsum("ij,ij->i", q, q)
    tn = np.einsum("ij,ij->i", tk, tk)[rows_safe]
    dots = np.einsum("bd,bkd->bk", q, tc_)
    d2 = qn[:, None] - 2.0 * dots + tn
    d2 = np.where(invalid, np.float32(np.inf), d2).astype(np.float32)

    top_k = np.argpartition(d2, K, axis=1)[:, :K]
    rows_k = np.take_along_axis(rows_safe, top_k, axis=1)

    # ---- reference tail: exact sq, inverse-distance weights ----
    nb = tk[rows_k]
    sq = np.sum((q[:, None, :] - nb) ** 2, axis=2, dtype=np.float32)
    w = np.float32(1.0) / (sq + np.float32(DELTA))
    w = w / np.sum(w, axis=1, keepdims=True)
    out = np.sum(w * v[rows_k], axis=1)
    return out.astype(np.float32)


# revision 31
# speedup vs baseline: 1.3322x; 1.0022x over previous
"""Distributed brute-force kNN (top-50 inverse-distance-weighted regression), v4.

Strategy (8 NeuronCores):
  - Table (500k x 64) norm-sorted and striped across 8 cores (62500 rows each,
    padded to 63488 = 62 units x 1024 cols). fp8e4m3 everywhere on device;
    dots via DoubleRow matmuls (K packed as [32 partitions x 2 k-tiles]).
  - Octet-major device layout: within a unit, device col = rank (= octet*8 +
    member). Per 128-query group the 62 units' PSUM dots drain through the
    only two engines that can legally read PSUM (walrus allows ONE PSUM
    operand per instruction; Pool/DMA cannot touch PSUM at all, and Pool has
    no working tensor ops in this toolchain):
      D (29/62): DVE tensor_reduce straight from PSUM -- full octet max to
        128 fp8 gm cols per unit (1192ns). Unit 61 is half padding (ranks
        >= 62500), so it runs one matmul and a half-width reduce (658ns).
      A (33/62): ACT evicts PSUM straight to 1024 raw fp8 gm cols per unit
        (1038ns); the host maxes the 8 members per octet.
    Both engines run ~94% busy; this split balances them.
  - gm staging tiles hold a PAIR of qgs ([P, 2*(W+1)] with a pad col per
    half) so the 16 DMA procs (8 hwdge + 8 swdge) suffice: every DMA sits on
    a fresh proc and carries at most ONE wait (its writer RAW; walrus allows
    a single semaphore wait per instruction). hwdge: 4 input chunks + 3
    D-pair ships + the qg6 D half; swdge: 3 A-pair ships + the qg6 A half +
    qg7 tail chunks (3 for A, smallest last, + the whole D block).
  - Host: rank octets by the distance lower bound min_norm - 2*gmax, take
    top N_GROUPS=256 octets, re-score the 8*N_GROUPS candidates exactly in
    f32, final top-50 + inverse-distance weights (reference formula).

Sync discipline: per-engine total order via nosync deps (link()); every
cross-engine hazard is carried by exactly one instruction wait. Absorber ops
pull engine clocks forward so Tile never emits a second wait: standalone PE
ldweights absorb PSUM-slot WARs and input-chunk RAWs; at pair-tile recycling
boundaries a PUMP (reads a cell the engine wrote last qg; one self-sem wait
pulling the engine's observed self-clock current) precedes the tile
allocation and a CATCHER (writes the pad col the recycled tile's ship read,
sourced from a scratch cell written once at qg0) takes the ship-DMA WAR as
its only wait.
"""

import numpy as np
import ml_dtypes

import concourse.bass as bass
import concourse.tile as tile
from concourse import mybir
from concourse.bass_utils import run_bass_kernel_spmd
from contextlib import ExitStack

# Problem geometry (hardcoded per spec).
B = 1024          # queries
D = 64            # feature dim
C = 500000        # table capacity
K = 50            # neighbours
DELTA = 1e-3

CORES = 8
N_TILE = 512      # matmul free dim (one PSUM bank)
PAIR_W = 2 * N_TILE
N_UNITS = 62      # 62 * 1024 = 63488 >= 62500 real rows per core
N_COLS = N_UNITS * PAIR_W
C_LOC = 63488     # table width in tq (= N_COLS exactly)
G = 8             # octet size
OCT_W = PAIR_W // G            # 128 octets per unit
N_OCT = N_UNITS * OCT_W        # 7936 octets per core
P = 128           # partition dim == query-group size
QG = B // P       # 8 query groups

N_GROUPS = 256    # candidate octets per query taken on host

# unit kinds: D = DVE tensor_reduce straight from PSUM (full octet fold to
# 128 cols; walrus allows just ONE PSUM operand per instruction, so the
# two-input PSUM folds are illegal); A = ACT evicts PSUM straight to fp8 gm
# cols (1024/unit; the host maxes the 8 members -- Pool/gpsimd has no
# working tensor ops in this toolchain, so there is no cheap fold stage).
# Counts balance DVE (1192ns/unit) vs ACT (1038ns/unit).
N_D = 29
N_A = 33


def _mk_units(n_d=N_D, n_a=N_A):
    # largest-remainder interleave for smooth engine load
    counts = {k: v for k, v in (("D", n_d), ("A", n_a)) if v > 0}
    acc = {k: 0.0 for k in counts}
    out = []
    for i in range(N_UNITS):
        for k in counts:
            acc[k] += counts[k] / N_UNITS
        k = max(acc, key=lambda kk: acc[kk])
        acc[k] -= 1.0
        out.append(k)
    i0 = out.index("D")
    out = out[i0:] + out[:i0]
    assert all(out.count(k) == counts.get(k, 0) for k in ("D", "A"))
    return out


UNITS = _mk_units()
# rotate so the sequence STARTS with an A unit: ACT is the binding engine,
# so its first evict should be fed as early as possible
UNITS = UNITS[1:] + UNITS[:1]
# unit 61 must be a D unit (its half-width saving lands on the binding DVE)
if UNITS[61] != "D":
    _j = max(i for i, k in enumerate(UNITS) if k == "D")
    UNITS[61], UNITS[_j] = UNITS[_j], UNITS[61]
DB_W = (N_D - 1) * OCT_W + OCT_W // 2 + 1   # D block + pad (u61 is half)
AB_W = N_A * PAIR_W + 1                 # A block + pad col (per qg half)
GM_W = DB_W + AB_W                      # DRAM gm row width
GM_DT = mybir.dt.float8e4
GM_NP_DT = ml_dtypes.float8_e4m3

# qg7 tail ship boundaries (A-unit index); the final chunk is smallest so
# the drain tail is short. The D block is small enough to ship whole.
A_TAIL_AT = (18, 29, N_A)

# quadrant layout: table third qt (21 units) lives on partitions
# [32qt, 32qt+32) with its own query replica (PE row-group bases 0/32/64;
# base 96 = broken HW quadrant).
UQ = 21
NQUAD = 3
NT_Q = 2 + 2 * UQ                  # 44 512-col tiles per third (2 = queries)

# input chunks (SP hwdge, procs 0-3): (part_lo, part_hi, tile_lo, tile_hi,
# first unit that needs it). Chunk 0 = queries + unit 0 only so compute
# starts as early as possible; thirds 1+2 merge into one 64-partition chunk
# to stay within 4 hwdge procs. The PE absorber for chunk k sits before
# unit need_u.
IN_CHUNKS = ((0, 32, 0, 6, 0), (0, 32, 6, 16, 2), (0, 32, 16, NT_Q, 7),
             (32, 96, 0, NT_Q, 21))

# unit 61 covers ranks [62464, 63488) but only 36 rows (< 62500) are real:
# its whole second 512-tile is padding, so it runs one matmul and a
# half-width tensor_reduce (658ns instead of 1192 on the binding DVE).
HALF_U = 61
HALF_OCT = OCT_W // 2

_NC_CACHE = {}


def _build_nc():
    if "nc" in _NC_CACHE:
        return _NC_CACHE["nc"]
    # swdge descriptor carveout sized so the ring never wraps (no ring waits)
    nc = bass.Bass(dynamic_dma_scratch_size=24576)
    # queries FIRST so the first weight loads only need the first DMA chunk
    # k-tiles interleaved per 512-col tile so every AP's k-step is 512
    tq = nc.declare_dram_parameter("tq", [96, NT_Q, 2, N_TILE],
                                   mybir.dt.float8e4, isOutput=False)
    gm = nc.declare_dram_parameter("gm", [B, GM_W], GM_DT, isOutput=True)

    from concourse.bass import _add_dep_helper as dep

    # Split TileContext's exit drain (one wait per active proc) into
    # one-wait-per-drain instructions.
    from concourse.vector_clock import VectorClock, ScopedClock

    def _split_drain_and_barrier(self, tick_clock, wait_clock):
        gc = tick_clock.global_clock
        for proc in range(27):
            t = gc.peek_next(proc) - 1
            if t <= 0:
                continue
            d = self.nc.sync.drain()
            pc = VectorClock()
            pc.require_at_least(proc, t)
            wait_clock.add_sem_waits(d.ins, ScopedClock({None: pc}))
        self.nc.all_engine_barrier()
        assert self.sems is not None
        popped = self.nc._tile_sem_poison_stack.pop()
        assert popped is self._sem_poison
        self.nc.clear_and_free_semaphores(list(self.sems.allocated().values()))
        self.nc.all_engine_barrier()

    tile.TileContext._drain_and_barrier = _split_drain_and_barrier

    with ExitStack() as ctx:
        tc = ctx.enter_context(tile.TileContext(nc, pool_alloc_mode="stack"))
        singles = ctx.enter_context(tc.tile_pool(name="singles", bufs=1))
        # one PSUM pool per consumer engine: within a tag all releases come
        # from one engine in issue order, so slot reuse is deterministic
        ppoolD = ctx.enter_context(tc.tile_pool(name="ppoolD", bufs=2, space="PSUM"))
        ppoolA = ctx.enter_context(tc.tile_pool(name="ppoolA", bufs=2, space="PSUM"))
        gmpoolD = ctx.enter_context(tc.tile_pool(name="gmpoolD", bufs=2))
        gmpoolA = ctx.enter_context(tc.tile_pool(name="gmpoolA", bufs=2))
        apool = ctx.enter_context(tc.tile_pool(name="apool", bufs=2))

        # chunked table load on the SP hwdge ring (procs 0-3): SP has no
        # engine work, hwdge descriptor gen is off the compute engines, and
        # the swdge procs stay fresh for the A/tail ships.
        t_sb = singles.tile([96, NT_Q, 2, N_TILE], mybir.dt.float8e4)
        for (pl, ph, tl, th, _nu) in IN_CHUNKS:
            nc.sync.dma_start(out=t_sb[pl:ph, tl:th], in_=tq[pl:ph, tl:th])

        chain = {"PE": None, "DVE": None, "ACT": None, "POOL": None}

        def link(key, inst):
            # total order per engine queue (add_dep_helper(a, b) == a waits b)
            if chain[key] is not None:
                dep(inst.ins, chain[key].ins, sync=False, reason=f"{key} order")
            chain[key] = inst
            return inst

        dscr = apool.tile([1, 1], GM_DT, tag="dscr")    # DVE pump sink
        ascr = apool.tile([1, 1], GM_DT, tag="ascr")    # ACT pump sink
        # catcher sources, written ONCE at qg0: later reads are ancient
        # same-engine RAWs already covered by the pump's self-clock
        dzero = apool.tile([1, 1], GM_DT, tag="dzero")
        azero4 = apool.tile([1, 4], GM_DT, tag="azero")
        azero = azero4[0:1, 0:1]
        link("DVE", nc.vector.memset(dzero, 0.0))
        link("ACT", nc.scalar.memzero(azero4))

        d_srcs = []            # gm cell AP per D-unit (PE absorber source)
        a_srcs = []            # gm cell AP per A-unit (PE absorber source)
        nd = 0                 # global D-unit counter
        na = 0                 # global A-unit counter
        gmD = gmA = None

        ck_i = 0
        for qg in range(QG):
            qq = (qg % 4) * P
            half = qg % 2
            dh = half * DB_W            # col base of this qg's half in gmD
            ah = half * AB_W            # col base of this qg's half in gmA
            if half == 0:
                if qg >= 4:
                    # pair-tile recycling: PUMP (reads the previous pair
                    # tile's odd-half cell -- recent self-RAW pulling the
                    # engine's self-clock to the present), then allocate, then
                    # CATCHER (takes the recycled tile's pair-ship WAR via the
                    # pad col -- its single wait).
                    link("DVE", nc.vector.tensor_copy(
                        dscr, gmD[0:1, DB_W:DB_W + 1]))
                    link("ACT", nc.scalar.copy(
                        ascr, gmA[0:1, AB_W:AB_W + 1]))
                    gmD = gmpoolD.tile([P, 2 * DB_W], GM_DT, tag="gmD")
                    link("DVE", nc.vector.tensor_copy(
                        gmD[0:1, DB_W - 1:DB_W], dzero))
                    gmA = gmpoolA.tile([P, 2 * AB_W], GM_DT, tag="gmA")
                    link("ACT", nc.scalar.copy(
                        gmA[0:1, AB_W - 1:AB_W], azero))
                else:
                    gmD = gmpoolD.tile([P, 2 * DB_W], GM_DT, tag="gmD")
                    gmA = gmpoolA.tile([P, 2 * AB_W], GM_DT, tag="gmA")

            di = 0             # D-units completed within this qg
            ai = 0             # A-units completed within this qg
            a_tail_i = 0
            for u, kind in enumerate(UNITS):
                qt = u // UQ            # third (partition base 32*qt)
                ps = slice(32 * qt, 32 * (qt + 1))
                ut = 2 + 2 * (u % UQ)   # first 512-tile of unit u in third
                lhsT = t_sb[ps, qg // 4, :, qq:qq + P]
                if qg == 0 and ck_i < len(IN_CHUNKS) and u == IN_CHUNKS[ck_i][4]:
                    # absorb this chunk's DMA-complete wait into a ldweights
                    cpl, cph, ctl, cth, _nu = IN_CHUNKS[ck_i]
                    link("PE", nc.tensor.ldweights(
                        weights=t_sb[cph - 32:cph, cth - 1, :, N_TILE - 1:N_TILE],
                        tile_position=(cph - 32, 0)))
                    ck_i += 1

                # PE absorber: standalone ldweights (no output; clobbered by
                # the next matmul's weight load) reading the output of the
                # consumer that released this unit's PSUM slot.
                if kind == "D":
                    dsrc = d_srcs[nd - 2] if nd >= 2 else t_sb[0:32, 0, 0, 0:1]
                else:
                    dsrc = a_srcs[na - 2] if na >= 2 else t_sb[0:32, 0, 0, 0:1]
                link("PE", nc.tensor.ldweights(weights=dsrc))

                pool_, tag_ = (ppoolD, "ppD") if kind == "D" else (ppoolA, "ppA")
                p = pool_.tile([P, PAIR_W], mybir.dt.float32, tag=tag_)
                nmm = 1 if u == HALF_U else 2
                for j in range(nmm):
                    link("PE", nc.tensor.matmul(
                        p[:, j * N_TILE:(j + 1) * N_TILE], lhsT,
                        t_sb[ps, ut + j], start=True, stop=True,
                        perf_mode=mybir.MatmulPerfMode.DoubleRow))

                pv = p.rearrange("p (o m) -> p o m", m=G)
                if kind == "D":
                    w = HALF_OCT if u == HALF_U else OCT_W
                    gbase = dh + di * OCT_W
                    link("DVE", nc.vector.tensor_reduce(
                        out=gmD[:, gbase:gbase + w], in_=pv[:, 0:w, :],
                        axis=mybir.AxisListType.X, op=mybir.AluOpType.max))
                    d_srcs.append(gmD[0:D, gbase:gbase + 1])
                    nd += 1
                    di += 1
                    if qg == QG - 1 and di == N_D:
                        # qg7 D block ships whole on a swdge proc
                        link("POOL", nc.gpsimd.dma_start(
                            out=gm[qg * P:(qg + 1) * P, 0:DB_W],
                            in_=gmD[:, dh:dh + DB_W]))
                else:
                    gbase = ah + ai * PAIR_W
                    link("ACT", nc.scalar.copy(
                        gmA[:, gbase:gbase + PAIR_W], p))
                    a_srcs.append(gmA[0:D, gbase:gbase + 1])
                    na += 1
                    ai += 1
                    if qg == QG - 1 and ai == A_TAIL_AT[a_tail_i]:
                        lo = (A_TAIL_AT[a_tail_i - 1] if a_tail_i else 0) * PAIR_W
                        hi = ai * PAIR_W if a_tail_i < len(A_TAIL_AT) - 1 else AB_W
                        link("POOL", nc.gpsimd.dma_start(
                            out=gm[qg * P:(qg + 1) * P, DB_W + lo:DB_W + hi],
                            in_=gmA[:, ah + lo:ah + hi]))
                        a_tail_i += 1

            if qg % 2 == 1 and qg < QG - 1:
                # pair ships (two qgs' rows in one DMA): D via SP hwdge, A via
                # the Pool swdge queue; one RAW wait each (single writer)
                rows = gm[(qg - 1) * P:(qg + 1) * P]
                nc.sync.dma_start(
                    out=rows[:, 0:DB_W].rearrange("(h p) c -> p h c", h=2),
                    in_=gmD.rearrange("p (h c) -> p h c", h=2))
                link("POOL", nc.gpsimd.dma_start(
                    out=rows[:, DB_W:GM_W].rearrange("(h p) c -> p h c", h=2),
                    in_=gmA.rearrange("p (h c) -> p h c", h=2)))
            elif qg == QG - 2:
                # qg6: ship this half now (qg7 ships at its end); D on the
                # last hwdge proc, A on a swdge proc
                nc.sync.dma_start(
                    out=gm[qg * P:(qg + 1) * P, 0:DB_W], in_=gmD[:, 0:DB_W])
                link("POOL", nc.gpsimd.dma_start(
                    out=gm[qg * P:(qg + 1) * P, DB_W:GM_W], in_=gmA[:, 0:AB_W]))

    # wait audit: walrus accepts at most one semaphore wait per instruction
    bad = []
    for blk in nc.m.functions[0].blocks:
        for inst in blk.instructions:
            si = inst.sync_info
            if si is None:
                continue
            nw = len(si.on_wait or [])
            if nw > 1:
                bad.append((type(inst).__name__, inst.name, nw))
    assert not bad, f"multi-wait instructions: {bad[:10]}"

    _NC_CACHE["nc"] = nc
    return nc


def _preprocess(table_keys):
    """Norm-sort, stripe across cores; octet-major device layout (device col
    within a unit = local rank, octet o covers ranks [8*(128u+o), ...+8)).
    gm holds 4 cols per octet (member-pair maxima from fold1); the host maxes
    them. Returns per-core fp8 tables plus octet-indexed candidate rows /
    min-norms and the octet -> gm column map."""
    tk = np.ascontiguousarray(table_keys, dtype=np.float32)
    norms = np.einsum("ij,ij->i", tk, tk)
    order = np.argsort(norms, kind="stable")

    # gm col base per unit: D block cols [0, N_D*OCT_W), pad, A block
    colbase = np.zeros(N_UNITS, dtype=np.int64)
    kinds = []
    dnext, anext = 0, DB_W
    for u, kind in enumerate(UNITS):
        kinds.append(kind)
        if kind == "D":
            colbase[u] = dnext
            dnext += OCT_W
        else:
            colbase[u] = anext
            anext += PAIR_W

    # octet id (u*128+o) -> its gm columns: D units have one col per octet
    # (repeated 8x), A units have the 8 raw member cols (host maxes them)
    oct2cols = np.zeros((8, N_OCT), dtype=np.int64)
    o = np.arange(OCT_W)
    for u in range(N_UNITS):
        oid = u * OCT_W + o
        for j in range(8):
            if kinds[u] == "D":
                # u61's octets beyond its half width are padding (rank >=
                # 62500, masked invalid by gnorm_min); clamp their col
                oc = np.minimum(o, OCT_W // 2 - 1) if u == 61 else o
                oct2cols[j, oid] = colbase[u] + oc
            else:
                oct2cols[j, oid] = colbase[u] + 8 * o + j

    tts = []
    cand_rows = np.full((CORES, N_OCT, G), -1, dtype=np.int64)
    gnorm_min = np.full((CORES, N_OCT), np.float32(1e9), dtype=np.float32)

    n_loc = C // CORES             # 62500 real rows per core

    for m in range(CORES):
        rows_m = order[m::CORES]
        Tdev = np.zeros((N_COLS, D), dtype=np.float32)   # rank == device col
        Tdev[:n_loc] = tk[rows_m]
        nrm = np.full(N_COLS, np.float32(1e9), dtype=np.float32)
        nrm[:n_loc] = norms[rows_m]

        oid = np.arange(N_OCT)
        ranks = oid[:, None] * G + np.arange(G)[None, :]
        valid = ranks < n_loc
        ranks_c = np.minimum(ranks, n_loc - 1)
        cand_rows[m] = np.where(valid, rows_m[ranks_c], -1)
        gnorm_min[m] = np.where(valid[:, 0], nrm[ranks[:, 0]], np.float32(1e9))

        tts.append(Tdev.T.astype(ml_dtypes.float8_e4m3))   # [64, C_LOC]

    return tts, cand_rows, gnorm_min, oct2cols


def kernel(keys, table_keys, table_values):
    q = np.ascontiguousarray(keys, dtype=np.float32)
    tk = np.ascontiguousarray(table_keys, dtype=np.float32)
    v = np.ascontiguousarray(table_values, dtype=np.float32)

    tts, cand_rows, gnorm_min, oct2cols = _preprocess(tk)
    q8 = q.T.astype(ml_dtypes.float8_e4m3)                 # [64, B]
    # per third: [64, (2 + 42)*512 cols] -> [32, NT_Q, 2, 512]; thirds
    # stacked on the partition axis -> [96, NT_Q, 2, 512]
    tqs = []
    for m in range(CORES):
        full = np.zeros((96, NT_Q, 2, N_TILE), dtype=ml_dtypes.float8_e4m3)
        for qt in range(NQUAD):
            u0 = qt * UQ
            nu = min(UQ, N_UNITS - u0)
            cols = np.concatenate(
                [q8, tts[m][:, u0 * PAIR_W:(u0 + nu) * PAIR_W]], axis=1)
            nt = 2 + 2 * nu
            packed = cols.reshape(2, 32, nt, N_TILE).transpose(1, 2, 0, 3)
            full[32 * qt:32 * (qt + 1), :nt] = packed
        tqs.append(full)

    nc = _build_nc()
    in_maps = [{"tq": tqs[m]} for m in range(CORES)]
    res = run_bass_kernel_spmd(nc, in_maps, core_ids=list(range(CORES)))
    gmax = np.stack([np.asarray(r["gm"]).astype(np.float32)
                     for r in res.results])              # [8, B, GM_W]

    # ---- host stage 2: rank octets by distance lower bound ----
    gmax = np.nan_to_num(gmax, nan=-1e9, posinf=-1e9, neginf=-1e9)
    gmax_oct = gmax[:, :, oct2cols[0]]
    for j in range(1, 8):
        gmax_oct = np.maximum(gmax_oct, gmax[:, :, oct2cols[j]])  # [8,B,N_OCT]
    invalid_g = gnorm_min >= np.float32(1e9)         # [8, N_OCT]
    gmax_oct = np.where(invalid_g[:, None, :], np.float32(-1e9), gmax_oct)
    lb = gnorm_min[:, None, :] - 2.0 * gmax_oct      # [8, B, N_OCT]
    lb = lb.transpose(1, 0, 2).reshape(B, CORES * N_OCT)
    top_g = np.argpartition(lb, N_GROUPS, axis=1)[:, :N_GROUPS]

    core_of = top_g // N_OCT
    g_of = top_g % N_OCT
    rows = cand_rows[core_of, g_of].reshape(B, N_GROUPS * G)
    invalid = rows < 0
    rows_safe = np.where(invalid, 0, rows)

    # ---- exact rescore with the reference's formula (f32) ----
    tc_ = tk[rows_safe]                               # [B, NCAND, D]
    qn = np.einsum("ij,ij->i", q, q)
    tn = np.einsum("ij,ij->i", tk, tk)[rows_safe]
    dots = np.einsum("bd,bkd->bk", q, tc_)
    d2 = qn[:, None] - 2.0 * dots + tn
    d2 = np.where(invalid, np.float32(np.inf), d2).astype(np.float32)

    top_k = np.argpartition(d2, K, axis=1)[:, :K]
    rows_k = np.take_along_axis(rows_safe, top_k, axis=1)

    # ---- reference tail: exact sq, inverse-distance weights ----
    nb = tk[rows_k]
    sq = np.sum((q[:, None, :] - nb) ** 2, axis=2, dtype=np.float32)
    w = np.float32(1.0) / (sq + np.float32(DELTA))
    w = w / np.sum(w, axis=1, keepdims=True)
    out = np.sum(w * v[rows_k], axis=1)
    return out.astype(np.float32)


# revision 34
# speedup vs baseline: 1.3329x; 1.0005x over previous
"""Distributed brute-force kNN (top-50 inverse-distance-weighted regression), v4.

Strategy (8 NeuronCores):
  - Table (500k x 64) norm-sorted and striped across 8 cores (62500 rows each,
    padded to 63488 = 62 units x 1024 cols). fp8e4m3 everywhere on device;
    dots via DoubleRow matmuls (K packed as [32 partitions x 2 k-tiles]).
  - Octet-major device layout: within a unit, device col = rank (= octet*8 +
    member). Per 128-query group the 62 units' PSUM dots drain through the
    only two engines that can legally read PSUM (walrus allows ONE PSUM
    operand per instruction; Pool/DMA cannot touch PSUM at all, and Pool has
    no working tensor ops in this toolchain):
      D (29/62): DVE tensor_reduce straight from PSUM -- full octet max to
        128 fp8 gm cols per unit (1192ns). Unit 61 is half padding (ranks
        >= 62500), so it runs one matmul and a half-width reduce (658ns).
      A (33/62): ACT evicts PSUM straight to 1024 raw fp8 gm cols per unit
        (1038ns); the host maxes the 8 members per octet.
    Both engines run ~94% busy; this split balances them.
  - gm staging tiles hold a PAIR of qgs ([P, 2*(W+1)] with a pad col per
    half) so the 16 DMA procs (8 hwdge + 8 swdge) suffice: every DMA sits on
    a fresh proc and carries at most ONE wait (its writer RAW; walrus allows
    a single semaphore wait per instruction). hwdge: 4 input chunks + 3
    D-pair ships + the qg6 D half; swdge: 3 A-pair ships + the qg6 A half +
    qg7 tail chunks (3 for A, smallest last, + the whole D block).
  - Host: rank octets by the distance lower bound min_norm - 2*gmax, take
    top N_GROUPS=256 octets, re-score the 8*N_GROUPS candidates exactly in
    f32, final top-50 + inverse-distance weights (reference formula).

Sync discipline: per-engine total order via nosync deps (link()); every
cross-engine hazard is carried by exactly one instruction wait. Absorber ops
pull engine clocks forward so Tile never emits a second wait: standalone PE
ldweights absorb PSUM-slot WARs and input-chunk RAWs; at pair-tile recycling
boundaries a PUMP (reads a cell the engine wrote last qg; one self-sem wait
pulling the engine's observed self-clock current) precedes the tile
allocation and a CATCHER (writes the pad col the recycled tile's ship read,
sourced from a scratch cell written once at qg0) takes the ship-DMA WAR as
its only wait.
"""

import numpy as np
import ml_dtypes

import concourse.bass as bass
import concourse.tile as tile
from concourse import mybir
from concourse.bass_utils import run_bass_kernel_spmd
from contextlib import ExitStack

# Problem geometry (hardcoded per spec).
B = 1024          # queries
D = 64            # feature dim
C = 500000        # table capacity
K = 50            # neighbours
DELTA = 1e-3

CORES = 8
N_TILE = 512      # matmul free dim (one PSUM bank)
PAIR_W = 2 * N_TILE
N_UNITS = 62      # 62 * 1024 = 63488 >= 62500 real rows per core
N_COLS = N_UNITS * PAIR_W
C_LOC = 63488     # table width in tq (= N_COLS exactly)
G = 8             # octet size
OCT_W = PAIR_W // G            # 128 octets per unit
N_OCT = N_UNITS * OCT_W        # 7936 octets per core
P = 128           # partition dim == query-group size
QG = B // P       # 8 query groups

N_GROUPS = 256    # candidate octets per query taken on host

# unit kinds: D = DVE tensor_reduce straight from PSUM (full octet fold to
# 128 cols; walrus allows just ONE PSUM operand per instruction, so the
# two-input PSUM folds are illegal); A = ACT evicts PSUM straight to fp8 gm
# cols (1024/unit; the host maxes the 8 members -- Pool/gpsimd has no
# working tensor ops in this toolchain, so there is no cheap fold stage).
# Counts balance DVE (1192ns/unit) vs ACT (1038ns/unit).
N_D = 29
N_A = 33


def _mk_units(n_d=N_D, n_a=N_A):
    # largest-remainder interleave for smooth engine load
    counts = {k: v for k, v in (("D", n_d), ("A", n_a)) if v > 0}
    acc = {k: 0.0 for k in counts}
    out = []
    for i in range(N_UNITS):
        for k in counts:
            acc[k] += counts[k] / N_UNITS
        k = max(acc, key=lambda kk: acc[kk])
        acc[k] -= 1.0
        out.append(k)
    i0 = out.index("D")
    out = out[i0:] + out[:i0]
    assert all(out.count(k) == counts.get(k, 0) for k in ("D", "A"))
    return out


UNITS = _mk_units()
# rotate so the sequence STARTS with an A unit: ACT is the binding engine,
# so its first evict should be fed as early as possible
UNITS = UNITS[1:] + UNITS[:1]
# unit 61 must be a D unit (its half-width saving lands on the binding DVE)
if UNITS[61] != "D":
    _j = max(i for i, k in enumerate(UNITS) if k == "D")
    UNITS[61], UNITS[_j] = UNITS[_j], UNITS[61]
DB_W = (N_D - 1) * OCT_W + OCT_W // 2 + 1   # D block + pad (u61 is half)
AB_W = N_A * PAIR_W + 1                 # A block + pad col (per qg half)
GM_W = DB_W + AB_W                      # DRAM gm row width
GM_DT = mybir.dt.float8e4
GM_NP_DT = ml_dtypes.float8_e4m3

# qg7 tail ship boundaries (A-unit index); the final chunk is smallest so
# the drain tail is short. The D block is small enough to ship whole.
A_TAIL_AT = (18, 29, N_A)

# quadrant layout: table third qt (21 units) lives on partitions
# [32qt, 32qt+32) with its own query replica (PE row-group bases 0/32/64;
# base 96 = broken HW quadrant).
UQ = 21
NQUAD = 3
NT_Q = 2 + 2 * UQ                  # 44 512-col tiles per third (2 = queries)

# input chunks (SP hwdge, procs 0-3): (part_lo, part_hi, tile_lo, tile_hi,
# first unit that needs it). Chunk 0 = queries + unit 0 only so compute
# starts as early as possible; thirds 1+2 merge into one 64-partition chunk
# to stay within 4 hwdge procs. The PE absorber for chunk k sits before
# unit need_u.
IN_CHUNKS = ((0, 32, 0, 6, 0), (0, 32, 6, 16, 2), (0, 32, 16, NT_Q, 7),
             (32, 96, 0, NT_Q, 21))

# unit 61 covers ranks [62464, 63488) but only 36 rows (< 62500) are real:
# its whole second 512-tile is padding, so it runs one matmul and a
# half-width tensor_reduce (658ns instead of 1192 on the binding DVE).
HALF_U = 61
HALF_OCT = OCT_W // 2

_NC_CACHE = {}


def _build_nc():
    if "nc" in _NC_CACHE:
        return _NC_CACHE["nc"]
    # swdge descriptor carveout sized so the ring never wraps (no ring waits)
    nc = bass.Bass(dynamic_dma_scratch_size=24576)
    # queries FIRST so the first weight loads only need the first DMA chunk
    # k-tiles interleaved per 512-col tile so every AP's k-step is 512
    tq = nc.declare_dram_parameter("tq", [96, NT_Q, 2, N_TILE],
                                   mybir.dt.float8e4, isOutput=False)
    gm = nc.declare_dram_parameter("gm", [B, GM_W], GM_DT, isOutput=True)

    from concourse.bass import _add_dep_helper as dep

    # Split TileContext's exit drain (one wait per active proc) into
    # one-wait-per-drain instructions.
    from concourse.vector_clock import VectorClock, ScopedClock

    def _split_drain_and_barrier(self, tick_clock, wait_clock):
        gc = tick_clock.global_clock
        for proc in range(27):
            t = gc.peek_next(proc) - 1
            if t <= 0:
                continue
            d = self.nc.sync.drain()
            pc = VectorClock()
            pc.require_at_least(proc, t)
            wait_clock.add_sem_waits(d.ins, ScopedClock({None: pc}))
        self.nc.all_engine_barrier()
        assert self.sems is not None
        popped = self.nc._tile_sem_poison_stack.pop()
        assert popped is self._sem_poison
        self.nc.clear_and_free_semaphores(list(self.sems.allocated().values()))
        self.nc.all_engine_barrier()

    tile.TileContext._drain_and_barrier = _split_drain_and_barrier

    with ExitStack() as ctx:
        tc = ctx.enter_context(tile.TileContext(nc, pool_alloc_mode="stack"))
        singles = ctx.enter_context(tc.tile_pool(name="singles", bufs=1))
        # one PSUM pool per consumer engine: within a tag all releases come
        # from one engine in issue order, so slot reuse is deterministic
        ppoolD = ctx.enter_context(tc.tile_pool(name="ppoolD", bufs=2, space="PSUM"))
        ppoolA = ctx.enter_context(tc.tile_pool(name="ppoolA", bufs=2, space="PSUM"))
        gmpoolD = ctx.enter_context(tc.tile_pool(name="gmpoolD", bufs=2))
        gmpoolA = ctx.enter_context(tc.tile_pool(name="gmpoolA", bufs=2))
        apool = ctx.enter_context(tc.tile_pool(name="apool", bufs=2))

        # chunked table load on the SP hwdge ring (procs 0-3): SP has no
        # engine work, hwdge descriptor gen is off the compute engines, and
        # the swdge procs stay fresh for the A/tail ships.
        t_sb = singles.tile([96, NT_Q, 2, N_TILE], mybir.dt.float8e4)
        for (pl, ph, tl, th, _nu) in IN_CHUNKS:
            nc.sync.dma_start(out=t_sb[pl:ph, tl:th], in_=tq[pl:ph, tl:th])

        chain = {"PE": None, "DVE": None, "ACT": None, "POOL": None}

        def link(key, inst):
            # total order per engine queue (add_dep_helper(a, b) == a waits b)
            if chain[key] is not None:
                dep(inst.ins, chain[key].ins, sync=False, reason=f"{key} order")
            chain[key] = inst
            return inst

        dscr = apool.tile([1, 1], GM_DT, tag="dscr")    # DVE pump sink
        ascr = apool.tile([1, 1], GM_DT, tag="ascr")    # ACT pump sink
        # catcher sources, written ONCE at qg0: later reads are ancient
        # same-engine RAWs already covered by the pump's self-clock
        dzero = apool.tile([1, 1], GM_DT, tag="dzero")
        azero4 = apool.tile([1, 4], GM_DT, tag="azero")
        azero = azero4[0:1, 0:1]
        link("DVE", nc.vector.memset(dzero, 0.0))
        link("ACT", nc.scalar.memzero(azero4))

        d_srcs = []            # gm cell AP per D-unit (PE absorber source)
        a_srcs = []            # gm cell AP per A-unit (PE absorber source)
        nd = 0                 # global D-unit counter
        na = 0                 # global A-unit counter
        gmD = gmA = None

        ck_i = 0
        for qg in range(QG):
            qq = (qg % 4) * P
            half = qg % 2
            dh = half * DB_W            # col base of this qg's half in gmD
            ah = half * AB_W            # col base of this qg's half in gmA
            if half == 0:
                if qg >= 4:
                    # pair-tile recycling: PUMP (reads the previous pair
                    # tile's odd-half cell -- recent self-RAW pulling the
                    # engine's self-clock to the present), then allocate, then
                    # CATCHER (takes the recycled tile's pair-ship WAR via the
                    # pad col -- its single wait).
                    link("DVE", nc.vector.tensor_copy(
                        dscr, gmD[0:1, DB_W:DB_W + 1]))
                    link("ACT", nc.scalar.copy(
                        ascr, gmA[0:1, AB_W:AB_W + 1]))
                    gmD = gmpoolD.tile([P, 2 * DB_W], GM_DT, tag="gmD")
                    link("DVE", nc.vector.tensor_copy(
                        gmD[0:1, DB_W - 1:DB_W], dzero))
                    gmA = gmpoolA.tile([P, 2 * AB_W], GM_DT, tag="gmA")
                    link("ACT", nc.scalar.copy(
                        gmA[0:1, AB_W - 1:AB_W], azero))
                else:
                    gmD = gmpoolD.tile([P, 2 * DB_W], GM_DT, tag="gmD")
                    gmA = gmpoolA.tile([P, 2 * AB_W], GM_DT, tag="gmA")

            di = 0             # D-units completed within this qg
            ai = 0             # A-units completed within this qg
            a_tail_i = 0
            for u, kind in enumerate(UNITS):
                qt = u // UQ            # third (partition base 32*qt)
                ps = slice(32 * qt, 32 * (qt + 1))
                ut = 2 + 2 * (u % UQ)   # first 512-tile of unit u in third
                lhsT = t_sb[ps, qg // 4, :, qq:qq + P]
                if qg == 0 and ck_i < len(IN_CHUNKS) and u == IN_CHUNKS[ck_i][4]:
                    # absorb this chunk's DMA-complete wait into a ldweights
                    cpl, cph, ctl, cth, _nu = IN_CHUNKS[ck_i]
                    link("PE", nc.tensor.ldweights(
                        weights=t_sb[cph - 32:cph, cth - 1, :, N_TILE - 1:N_TILE],
                        tile_position=(cph - 32, 0)))
                    ck_i += 1

                # PE absorber: standalone ldweights (no output; clobbered by
                # the next matmul's weight load) reading the output of the
                # consumer that released this unit's PSUM slot.
                if kind == "D":
                    dsrc = d_srcs[nd - 2] if nd >= 2 else t_sb[0:32, 0, 0, 0:1]
                else:
                    dsrc = a_srcs[na - 2] if na >= 2 else t_sb[0:32, 0, 0, 0:1]
                link("PE", nc.tensor.ldweights(weights=dsrc))

                pool_, tag_ = (ppoolD, "ppD") if kind == "D" else (ppoolA, "ppA")
                p = pool_.tile([P, PAIR_W], mybir.dt.float32, tag=tag_)
                nmm = 1 if u == HALF_U else 2
                for j in range(nmm):
                    link("PE", nc.tensor.matmul(
                        p[:, j * N_TILE:(j + 1) * N_TILE], lhsT,
                        t_sb[ps, ut + j], start=True, stop=True,
                        perf_mode=mybir.MatmulPerfMode.DoubleRow))

                pv = p.rearrange("p (o m) -> p o m", m=G)
                if kind == "D":
                    w = HALF_OCT if u == HALF_U else OCT_W
                    gbase = dh + di * OCT_W
                    link("DVE", nc.vector.tensor_reduce(
                        out=gmD[:, gbase:gbase + w], in_=pv[:, 0:w, :],
                        axis=mybir.AxisListType.X, op=mybir.AluOpType.max))
                    d_srcs.append(gmD[0:D, gbase:gbase + 1])
                    nd += 1
                    di += 1
                    if qg == QG - 1 and di == N_D:
                        # qg7 D block ships whole on the last hwdge proc
                        # (faster dispatch on the critical end chain)
                        nc.sync.dma_start(
                            out=gm[qg * P:(qg + 1) * P, 0:DB_W],
                            in_=gmD[:, dh:dh + DB_W])
                else:
                    gbase = ah + ai * PAIR_W
                    link("ACT", nc.scalar.copy(
                        gmA[:, gbase:gbase + PAIR_W], p))
                    a_srcs.append(gmA[0:D, gbase:gbase + 1])
                    na += 1
                    ai += 1
                    if qg == QG - 1 and ai == A_TAIL_AT[a_tail_i]:
                        lo = (A_TAIL_AT[a_tail_i - 1] if a_tail_i else 0) * PAIR_W
                        hi = ai * PAIR_W if a_tail_i < len(A_TAIL_AT) - 1 else AB_W
                        link("POOL", nc.gpsimd.dma_start(
                            out=gm[qg * P:(qg + 1) * P, DB_W + lo:DB_W + hi],
                            in_=gmA[:, ah + lo:ah + hi]))
                        a_tail_i += 1

            if qg % 2 == 1 and qg < QG - 1:
                # pair ships (two qgs' rows in one DMA): D via SP hwdge, A via
                # the Pool swdge queue; one RAW wait each (single writer)
                rows = gm[(qg - 1) * P:(qg + 1) * P]
                nc.sync.dma_start(
                    out=rows[:, 0:DB_W].rearrange("(h p) c -> p h c", h=2),
                    in_=gmD.rearrange("p (h c) -> p h c", h=2))
                link("POOL", nc.gpsimd.dma_start(
                    out=rows[:, DB_W:GM_W].rearrange("(h p) c -> p h c", h=2),
                    in_=gmA.rearrange("p (h c) -> p h c", h=2)))
            elif qg == QG - 2:
                # qg6: ship this half now (qg7 ships at its end); D on a
                # swdge proc (mid-run, latency-insensitive), A on swdge too
                link("POOL", nc.gpsimd.dma_start(
                    out=gm[qg * P:(qg + 1) * P, 0:DB_W], in_=gmD[:, 0:DB_W]))
                link("POOL", nc.gpsimd.dma_start(
                    out=gm[qg * P:(qg + 1) * P, DB_W:GM_W], in_=gmA[:, 0:AB_W]))

    # wait audit: walrus accepts at most one semaphore wait per instruction
    bad = []
    for blk in nc.m.functions[0].blocks:
        for inst in blk.instructions:
            si = inst.sync_info
            if si is None:
                continue
            nw = len(si.on_wait or [])
            if nw > 1:
                bad.append((type(inst).__name__, inst.name, nw))
    assert not bad, f"multi-wait instructions: {bad[:10]}"

    _NC_CACHE["nc"] = nc
    return nc


def _preprocess(table_keys):
    """Norm-sort, stripe across cores; octet-major device layout (device col
    within a unit = local rank, octet o covers ranks [8*(128u+o), ...+8)).
    gm holds 4 cols per octet (member-pair maxima from fold1); the host maxes
    them. Returns per-core fp8 tables plus octet-indexed candidate rows /
    min-norms and the octet -> gm column map."""
    tk = np.ascontiguousarray(table_keys, dtype=np.float32)
    norms = np.einsum("ij,ij->i", tk, tk)
    order = np.argsort(norms, kind="stable")

    # gm col base per unit: D block cols [0, N_D*OCT_W), pad, A block
    colbase = np.zeros(N_UNITS, dtype=np.int64)
    kinds = []
    dnext, anext = 0, DB_W
    for u, kind in enumerate(UNITS):
        kinds.append(kind)
        if kind == "D":
            colbase[u] = dnext
            dnext += OCT_W
        else:
            colbase[u] = anext
            anext += PAIR_W

    # octet id (u*128+o) -> its gm columns: D units have one col per octet
    # (repeated 8x), A units have the 8 raw member cols (host maxes them)
    oct2cols = np.zeros((8, N_OCT), dtype=np.int64)
    o = np.arange(OCT_W)
    for u in range(N_UNITS):
        oid = u * OCT_W + o
        for j in range(8):
            if kinds[u] == "D":
                # u61's octets beyond its half width are padding (rank >=
                # 62500, masked invalid by gnorm_min); clamp their col
                oc = np.minimum(o, OCT_W // 2 - 1) if u == 61 else o
                oct2cols[j, oid] = colbase[u] + oc
            else:
                oct2cols[j, oid] = colbase[u] + 8 * o + j

    tts = []
    cand_rows = np.full((CORES, N_OCT, G), -1, dtype=np.int64)
    gnorm_min = np.full((CORES, N_OCT), np.float32(1e9), dtype=np.float32)

    n_loc = C // CORES             # 62500 real rows per core

    for m in range(CORES):
        rows_m = order[m::CORES]
        Tdev = np.zeros((N_COLS, D), dtype=np.float32)   # rank == device col
        Tdev[:n_loc] = tk[rows_m]
        nrm = np.full(N_COLS, np.float32(1e9), dtype=np.float32)
        nrm[:n_loc] = norms[rows_m]

        oid = np.arange(N_OCT)
        ranks = oid[:, None] * G + np.arange(G)[None, :]
        valid = ranks < n_loc
        ranks_c = np.minimum(ranks, n_loc - 1)
        cand_rows[m] = np.where(valid, rows_m[ranks_c], -1)
        gnorm_min[m] = np.where(valid[:, 0], nrm[ranks[:, 0]], np.float32(1e9))

        tts.append(Tdev.T.astype(ml_dtypes.float8_e4m3))   # [64, C_LOC]

    return tts, cand_rows, gnorm_min, oct2cols


def kernel(keys, table_keys, table_values):
    q = np.ascontiguousarray(keys, dtype=np.float32)
    tk = np.ascontiguousarray(table_keys, dtype=np.float32)
    v = np.ascontiguousarray(table_values, dtype=np.float32)

    tts, cand_rows, gnorm_min, oct2cols = _preprocess(tk)
    q8 = q.T.astype(ml_dtypes.float8_e4m3)                 # [64, B]
    # per third: [64, (2 + 42)*512 cols] -> [32, NT_Q, 2, 512]; thirds
    # stacked on the partition axis -> [96, NT_Q, 2, 512]
    tqs = []
    for m in range(CORES):
        full = np.zeros((96, NT_Q, 2, N_TILE), dtype=ml_dtypes.float8_e4m3)
        for qt in range(NQUAD):
            u0 = qt * UQ
            nu = min(UQ, N_UNITS - u0)
            cols = np.concatenate(
                [q8, tts[m][:, u0 * PAIR_W:(u0 + nu) * PAIR_W]], axis=1)
            nt = 2 + 2 * nu
            packed = cols.reshape(2, 32, nt, N_TILE).transpose(1, 2, 0, 3)
            full[32 * qt:32 * (qt + 1), :nt] = packed
        tqs.append(full)

    nc = _build_nc()
    in_maps = [{"tq": tqs[m]} for m in range(CORES)]
    res = run_bass_kernel_spmd(nc, in_maps, core_ids=list(range(CORES)))
    gmax = np.stack([np.asarray(r["gm"]).astype(np.float32)
                     for r in res.results])              # [8, B, GM_W]

    # ---- host stage 2: rank octets by distance lower bound ----
    gmax = np.nan_to_num(gmax, nan=-1e9, posinf=-1e9, neginf=-1e9)
    gmax_oct = gmax[:, :, oct2cols[0]]
    for j in range(1, 8):
        gmax_oct = np.maximum(gmax_oct, gmax[:, :, oct2cols[j]])  # [8,B,N_OCT]
    invalid_g = gnorm_min >= np.float32(1e9)         # [8, N_OCT]
    gmax_oct = np.where(invalid_g[:, None, :], np.float32(-1e9), gmax_oct)
    lb = gnorm_min[:, None, :] - 2.0 * gmax_oct      # [8, B, N_OCT]
    lb = lb.transpose(1, 0, 2).reshape(B, CORES * N_OCT)
    top_g = np.argpartition(lb, N_GROUPS, axis=1)[:, :N_GROUPS]

    core_of = top_g // N_OCT
    g_of = top_g % N_OCT
    rows = cand_rows[core_of, g_of].reshape(B, N_GROUPS * G)
    invalid = rows < 0
    rows_safe = np.where(invalid, 0, rows)

    # ---- exact rescore with the reference's formula (f32) ----
    tc_ = tk[rows_safe]                               # [B, NCAND, D]
    qn = np.einsum("ij,ij->i", q, q)
    tn = np.einsum("ij,ij->i", tk, tk)[rows_safe]
    dots = np.einsum("bd,bkd->bk", q, tc_)
    d2 = qn[:, None] - 2.0 * dots + tn
    d2 = np.where(invalid, np.float32(np.inf), d2).astype(np.float32)

    top_k = np.argpartition(d2, K, axis=1)[:, :K]
    rows_k = np.take_along_axis(rows_safe, top_k, axis=1)

    # ---- reference tail: exact sq, inverse-distance weights ----
    nb = tk[rows_k]
    sq = np.sum((q[:, None, :] - nb) ** 2, axis=2, dtype=np.float32)
    w = np.float32(1.0) / (sq + np.float32(DELTA))
    w = w / np.sum(w, axis=1, keepdims=True)
    out = np.sum(w * v[rows_k], axis=1)
    return out.astype(np.float32)
